# revision 8
# baseline (speedup 1.0000x reference)
"""Trainium2 Bass kernel for nn_LocalGlobalAttention (band + global-token attention).

Sharding: tensor-parallel over the 8 heads — one head per NeuronCore, no
collectives (each core computes its head's projections, attention and context;
the host concatenates).

On-device sparsity (exact, not approximate):
  - For each 128-query tile, scores are computed only on the 384-wide band
    strip plus the Ng global key columns. Softmax over that window equals the
    reference's full-row softmax because masked entries (-1e4 bias) underflow
    to exactly 0.0 in fp32.
  - Global-token query rows (Ng of them) get a dense softmax pass.
  - The global attention pass (Wqg/Wkg/Wvg) is evaluated only at the Ng global
    query positions — the only rows where its result is used.

Host side does layout transforms (hidden_states transpose, per-head weight
packing with the 1/sqrt(d) scale folded in), computes small mask-derived bias
tables, and scatters the compact device outputs into the full dense outputs.
"""
import os
import sys
from contextlib import ExitStack

for _p in ("/opt/trn_rl_repo", "/root/.axon_site/_ro/trn_rl_repo"):
    if os.path.isdir(_p) and _p not in sys.path:
        sys.path.insert(0, _p)

import numpy as np

import concourse.bass as bass
import concourse.tile as tile
from concourse import bacc, mybir
from concourse.bass_utils import run_bass_kernel_spmd

SEQ = 2048
DIM = 512
HEADS = 8
HD = 64          # head dim
WIN = 128        # local attention window
NEG = -10000.0
P = 128          # partitions
NT = SEQ // P    # 16 query tiles
KC = DIM // P    # 4 contraction chunks
NSC = SEQ // 512  # 4 sequence chunks of 512
SCALE = 1.0 / np.sqrt(HD)
F32 = mybir.dt.float32
F32R = mybir.dt.float32r
BF16 = mybir.dt.bfloat16

N_CORES = 8


def _strip_geom(t):
    """Key range covered by query tile t's band strip and its offset within
    the 384-wide strip window [128*(t-1), 128*(t+2))."""
    lo = max(0, P * (t - 1))
    hi = min(SEQ, P * (t + 2))
    off = lo - P * (t - 1)
    return lo, off, hi - lo


def _mask_tables(m):
    """Host-side mask preprocessing. m: (SEQ,) int32 (1 real, -1 global, 0 pad)."""
    is_pad = m == 0
    is_glob = m == -1
    globpos = np.flatnonzero(is_glob).astype(np.int64)
    ng = len(globpos)
    wtot = 384 + ng
    assert wtot <= 512, f"too many global tokens for strip layout: {ng}"

    gsec = np.where(is_pad[globpos], NEG, 0.0).astype(np.float32) if ng else np.zeros(0, np.float32)
    variants, vidx, vmap = [], [], {}
    for t in range(NT):
        i = P * t + np.arange(P)[:, None]
        j = P * (t - 1) + np.arange(384)[None, :]
        valid = (j >= 0) & (j < SEQ)
        jc = np.clip(j, 0, SEQ - 1)
        band = np.abs(i - j) <= WIN
        allowed = valid & band & ~is_pad[jc] & ~is_glob[jc]
        bias_t = np.empty((P, wtot), np.float32)
        bias_t[:, :384] = np.where(allowed, 0.0, NEG)
        if ng:
            bias_t[:, 384:] = gsec[None, :]
        key = bias_t.tobytes()
        if key not in vmap:
            vmap[key] = len(variants)
            variants.append(bias_t)
        vidx.append(vmap[key])
    biasv = np.stack(variants)                                   # (NV, P, wtot)
    gbias = np.broadcast_to(np.where(is_pad, NEG, 0.0).astype(np.float32), (max(ng, 1), SEQ)).copy()
    notpad = (~is_pad).astype(np.float32).reshape(NT, P).T.copy()  # (P, NT)
    return is_pad, is_glob, globpos, ng, wtot, biasv, vidx, gbias, notpad


def _build_program(ng, wtot, vidx, globpos, nvar):
    """Trace the per-core Bass/Tile program (SPMD: same program on all cores)."""
    nc = bacc.Bacc("TRN2", target_bir_lowering=False, debug=False, num_devices=1)

    hsT = nc.dram_tensor("hsT", (DIM, SEQ), F32, kind="ExternalInput").ap()
    wpack = nc.dram_tensor("wpack", (DIM, 384), F32, kind="ExternalInput").ap()
    bpack = nc.dram_tensor("bpack", (P, 3), F32, kind="ExternalInput").ap()
    biasv = nc.dram_tensor("biasv", (nvar, P, wtot), F32, kind="ExternalInput").ap()
    gbias = nc.dram_tensor("gbias", (max(ng, 1), SEQ), F32, kind="ExternalInput").ap()
    notpad = nc.dram_tensor("notpad", (P, NT), F32, kind="ExternalInput").ap()
    ident = nc.dram_tensor("ident", (P, P), F32, kind="ExternalInput").ap()

    pstrips = nc.dram_tensor("pstrips", (NT, P, wtot), F32, kind="ExternalOutput").ap()
    ctx_out = nc.dram_tensor("ctx", (SEQ, HD), F32, kind="ExternalOutput").ap()
    if ng:
        grow_out = nc.dram_tensor("growattn", (ng, SEQ), F32, kind="ExternalOutput").ap()
        gctx_out = nc.dram_tensor("gctx", (ng, HD), F32, kind="ExternalOutput").ap()

    uniform = False
    if ng >= 2:
        d = np.diff(globpos)
        uniform = len(set(d.tolist())) == 1
        g0, gst = int(globpos[0]), int(d[0])
    elif ng == 1:
        uniform, g0, gst = True, int(globpos[0]), 1

    with tile.TileContext(nc) as tc:
        with ExitStack() as ctx:
            consts = ctx.enter_context(tc.tile_pool(name="consts", bufs=1))
            ps_proj = ctx.enter_context(tc.tile_pool(name="ps_proj", bufs=2, space="PSUM"))
            ps_s = ctx.enter_context(tc.tile_pool(name="ps_s", bufs=2, space="PSUM"))
            ps_tr = ctx.enter_context(tc.tile_pool(name="ps_tr", bufs=2, space="PSUM"))
            ps_ctx = ctx.enter_context(tc.tile_pool(name="ps_ctx", bufs=2, space="PSUM"))

            # ---- load + round inputs (fp32 staging freed after rounding) ----
            staging = tc.tile_pool(name="staging", bufs=1)
            stg = staging.__enter__()
            hs_sb = stg.tile([P, KC, SEQ], F32)
            nc.sync.dma_start(hs_sb[:], hsT.rearrange("(c p) s -> p c s", p=P))
            w_sb = stg.tile([P, KC, 384], F32)
            nc.sync.dma_start(w_sb[:], wpack.rearrange("(c p) m -> p c m", p=P))
            bp_sb = consts.tile([P, 3], F32)
            nc.sync.dma_start(bp_sb[:], bpack)
            bv_sb = consts.tile([P, nvar, wtot], F32)
            nc.sync.dma_start(bv_sb[:], biasv.rearrange("v p w -> p v w"))
            gb_sb = consts.tile([max(ng, 1), SEQ], F32)
            nc.sync.dma_start(gb_sb[:], gbias)
            np_sb = consts.tile([P, NT], F32)
            nc.sync.dma_start(np_sb[:], notpad)
            id_sb = stg.tile([P, P], F32)
            nc.sync.dma_start(id_sb[:], ident)

            hs_r = consts.tile([P, KC, SEQ], F32R)
            w_r = consts.tile([P, KC, 384], F32R)
            for c in range(KC):
                nc.vector.tensor_copy(hs_r[:, c], hs_sb[:, c])
                nc.vector.tensor_copy(w_r[:, c], w_sb[:, c])
            id_r = consts.tile([P, P], F32R)
            nc.vector.tensor_copy(id_r[:], id_sb[:])
            staging.__exit__(None, None, None)

            work = ctx.enter_context(tc.tile_pool(name="work", bufs=3))
            ptp = ctx.enter_context(tc.tile_pool(name="ptp", bufs=6))
            vec = ctx.enter_context(tc.tile_pool(name="vec", bufs=4))

            # ---- projections: 3 packs of [2 x 64 features] over full seq ----
            # pack0 = [q|qg], pack1 = [k|kg], pack2 = [v|vg]; q/qg pre-scaled on host.
            proj = [consts.tile([P, SEQ], F32R, tag=f"proj{p}", name=f"proj{p}") for p in range(3)]
            for p in range(3):
                for s in range(NSC):
                    psa = ps_proj.tile([P, 512], F32, tag="proj")
                    for c in range(KC):
                        nc.tensor.matmul(psa[:], w_r[:, c, p * P:(p + 1) * P],
                                         hs_r[:, c, s * 512:(s + 1) * 512],
                                         start=(c == 0), stop=(c == KC - 1))
                    nc.vector.tensor_scalar(
                        out=proj[p][:, s * 512:(s + 1) * 512], in0=psa[:],
                        scalar1=bp_sb[:, p:p + 1], scalar2=None, op0=mybir.AluOpType.add)

            qT = proj[0][0:64, :]
            qgT = proj[0][64:128, :]
            kT = proj[1][0:64, :]
            kgT = proj[1][64:128, :]
            vT = proj[2][0:64, :]
            vgT = proj[2][64:128, :]

            def glob_cols(src):
                """(64, SEQ) AP -> (64, ng) AP of the global key columns."""
                if uniform:
                    return src[:, g0:g0 + gst * ng:gst]
                st = consts.tile([64, ng], F32R, tag=f"gcols{len(gcol_stages)}")
                gcol_stages.append(st)
                for k, pp in enumerate(globpos):
                    nc.vector.tensor_copy(st[:, k:k + 1].bitcast(F32),
                                          src[:, int(pp):int(pp) + 1].bitcast(F32))
                return st[:]
            gcol_stages = []

            # ---- V / Vg in (keys, 64) layout via PE transposes ----
            v_bf = consts.tile([P, NT, HD], F32R)
            vg_bf = consts.tile([P, NT, HD], F32R)
            for c in range(NT):
                pt = ps_tr.tile([P, P], F32R, tag="tr")
                nc.tensor.transpose(pt[0:P, 0:HD], vT[:, c * P:(c + 1) * P], id_r[0:64, 0:64])
                nc.scalar.copy(v_bf[:, c], pt[0:P, 0:HD].bitcast(F32))
            if ng:
                for c in range(NT):
                    pt = ps_tr.tile([P, P], F32R, tag="tr")
                    nc.tensor.transpose(pt[0:P, 0:HD], vgT[:, c * P:(c + 1) * P], id_r[64:128, 64:128])
                    nc.scalar.copy(vg_bf[:, c], pt[0:P, 0:HD].bitcast(F32))
                vglob_bf = consts.tile([ng, HD], F32R)
                pt = ps_tr.tile([P, P], F32R, tag="tr")
                nc.tensor.transpose(pt[0:ng, 0:HD], glob_cols(vT), id_r[0:64, 0:64])
                nc.scalar.copy(vglob_bf[:], pt[0:ng, 0:HD].bitcast(F32))
                kglob = glob_cols(kT)

            # ---- local attention, one 128-query tile at a time ----
            for t in range(NT):
                lo, off, wid = _strip_geom(t)
                v = vidx[t]
                sps = ps_s.tile([P, wtot], F32, tag="s")
                nc.tensor.matmul(sps[:, off:off + wid], qT[:, t * P:(t + 1) * P],
                                 kT[:, lo:lo + wid], start=True, stop=True)
                if ng:
                    nc.tensor.matmul(sps[:, 384:384 + ng], qT[:, t * P:(t + 1) * P],
                                     kglob, start=True, stop=True)
                s_sb = work.tile([P, wtot], F32, tag="ssb")
                if off > 0:
                    nc.vector.memset(s_sb[:, 0:off], NEG)
                end = off + wid
                if end == 384:   # strip and glob sections contiguous
                    nc.vector.tensor_add(s_sb[:, off:wtot], sps[:, off:wtot], bv_sb[:, v, off:wtot])
                else:
                    nc.vector.memset(s_sb[:, end:384], NEG)
                    nc.vector.tensor_add(s_sb[:, off:end], sps[:, off:end], bv_sb[:, v, off:end])
                    if ng:
                        nc.vector.tensor_add(s_sb[:, 384:wtot], sps[:, 384:wtot], bv_sb[:, v, 384:wtot])

                nm = vec.tile([P, 1], F32, tag="nm")
                nc.vector.reduce_max(nm[:], s_sb[:], axis=mybir.AxisListType.X, negate=True)
                p_sb = work.tile([P, wtot], F32R, tag="p")
                rs = vec.tile([P, 1], F32, tag="rs")
                nc.scalar.activation(p_sb[:], s_sb[:], mybir.ActivationFunctionType.Exp,
                                     bias=nm[:], scale=1.0, accum_out=rs[:])
                ri = vec.tile([P, 1], F32, tag="ri")
                nc.vector.reciprocal(ri[:], rs[:])
                rin = vec.tile([P, 1], F32, tag="rin")
                nc.vector.tensor_mul(rin[:], ri[:], np_sb[:, t:t + 1])

                pn = work.tile([P, wtot], F32, tag="pn")
                nc.scalar.mul(pn[:], p_sb[:].bitcast(F32), rin[:])
                nc.sync.dma_start(pstrips[t], pn[:])

                # context: ctx_t = P_strip @ V + P_glob @ Vglob, scaled by rin
                cps = ps_ctx.tile([P, HD], F32, tag="ctx")
                chunks = [c for c in range(3) if 0 <= t - 1 + c < NT]
                for i, c in enumerate(chunks):
                    ptt = ps_tr.tile([P, P], F32R, tag="tr")
                    nc.tensor.transpose(ptt[:], p_sb[:, c * P:(c + 1) * P], id_r[:])
                    pt_bf = ptp.tile([P, P], F32R, tag="ptbf")
                    nc.scalar.copy(pt_bf[:], ptt[:].bitcast(F32))
                    nc.tensor.matmul(cps[:], pt_bf[:], v_bf[:, t - 1 + c],
                                     start=(i == 0), stop=(i == len(chunks) - 1 and ng == 0))
                if ng:
                    ptt = ps_tr.tile([P, P], F32R, tag="tr")
                    nc.tensor.transpose(ptt[0:ng, :], p_sb[:, 384:384 + ng], id_r[:])
                    ptg_bf = ptp.tile([ng, P], F32R, tag="ptgbf")
                    nc.scalar.copy(ptg_bf[:], ptt[0:ng, :].bitcast(F32))
                    nc.tensor.matmul(cps[:], ptg_bf[:], vglob_bf[:], start=False, stop=True)
                ctx_sb = work.tile([P, HD], F32, tag="ctxsb")
                nc.scalar.mul(ctx_sb[:], cps[:], rin[:])
                nc.sync.dma_start(ctx_out[t * P:(t + 1) * P, :], ctx_sb[:])

            # ---- dense passes over the ng global query rows ----
            if ng:
                def dense_rows(qrows, kTfull):
                    """softmax(qrows.T @ kTfull + gbias) for ng query rows.
                    Returns (probs_f32r_tile, rinv_tile)."""
                    s16 = work.tile([ng, SEQ], F32, tag="s16", bufs=1)
                    for s in range(NSC):
                        gps = ps_s.tile([P, 512], F32, tag="s")
                        nc.tensor.matmul(gps[0:ng, :], qrows, kTfull[:, s * 512:(s + 1) * 512],
                                         start=True, stop=True)
                        nc.vector.tensor_add(s16[:, s * 512:(s + 1) * 512], gps[0:ng, :],
                                             gb_sb[:, s * 512:(s + 1) * 512])
                    nm16 = vec.tile([ng, 1], F32, tag="nm16")
                    nc.vector.reduce_max(nm16[:], s16[:], axis=mybir.AxisListType.X, negate=True)
                    p16 = work.tile([ng, SEQ], F32R, tag="p16", bufs=1)
                    rs16 = vec.tile([ng, 1], F32, tag="rs16")
                    nc.scalar.activation(p16[:], s16[:], mybir.ActivationFunctionType.Exp,
                                         bias=nm16[:], scale=1.0, accum_out=rs16[:])
                    ri16 = vec.tile([ng, 1], F32, tag="ri16")
                    nc.vector.reciprocal(ri16[:], rs16[:])
                    return p16, ri16

                # local attention rows at global positions (for the attn output)
                p16, ri16 = dense_rows(glob_cols(qT), kT)
                pn16 = work.tile([ng, SEQ], F32, tag="pn16", bufs=1)
                nc.scalar.mul(pn16[:], p16[:].bitcast(F32), ri16[:])
                nc.sync.dma_start(grow_out, pn16[:])

                # global attention pass (for the out rows at global positions)
                if uniform:
                    kg_mm = kgT
                else:
                    kg0 = consts.tile([64, SEQ], F32R)
                    nc.vector.tensor_copy(kg0[:].bitcast(F32), kgT.bitcast(F32))
                    kg_mm = kg0[:]
                pg16, rig16 = dense_rows(glob_cols(qgT), kg_mm)
                gps_ctx = ps_ctx.tile([P, HD], F32, tag="ctx")
                for c in range(NT):
                    ptt = ps_tr.tile([P, P], F32R, tag="tr")
                    nc.tensor.transpose(ptt[:, 0:ng], pg16[:, c * P:(c + 1) * P], id_r[0:ng, 0:ng])
                    pgt_bf = ptp.tile([P, ng], F32R, tag="pgtbf")
                    nc.scalar.copy(pgt_bf[:], ptt[:, 0:ng].bitcast(F32))
                    nc.tensor.matmul(gps_ctx[0:ng, :], pgt_bf[:], vg_bf[:, c],
                                     start=(c == 0), stop=(c == NT - 1))
                gctx_sb = work.tile([ng, HD], F32, tag="gctxsb")
                nc.scalar.mul(gctx_sb[:], gps_ctx[0:ng, :], rig16[:])
                nc.sync.dma_start(gctx_out, gctx_sb[:])

    nc.compile()
    return nc


_PROGRAM_CACHE = {}


def _get_program(ng, wtot, vidx, globpos, nvar):
    key = (ng, wtot, tuple(vidx), tuple(globpos.tolist()), nvar)
    if key not in _PROGRAM_CACHE:
        _PROGRAM_CACHE[key] = _build_program(ng, wtot, vidx, globpos, nvar)
    return _PROGRAM_CACHE[key]


def _pack_weights(Wq, bq, Wk, bk, Wv, bv, Wqg, bqg, Wkg, bkg, Wvg, bvg, h):
    """Per-head packed weight (DIM, 384) and bias (P, 3) with scale folded into q/qg."""
    sl = slice(h * HD, (h + 1) * HD)
    wpack = np.concatenate([Wq[sl].T * SCALE, Wqg[sl].T * SCALE,
                            Wk[sl].T, Wkg[sl].T,
                            Wv[sl].T, Wvg[sl].T], axis=1)
    bpack = np.stack([np.concatenate([bq[sl] * SCALE, bqg[sl] * SCALE]),
                      np.concatenate([bk[sl], bkg[sl]]),
                      np.concatenate([bv[sl], bvg[sl]])], axis=1)
    return np.ascontiguousarray(wpack, np.float32), np.ascontiguousarray(bpack, np.float32)


def make_in_maps(hidden_states, Wq, bq, Wk, bk, Wv, bv, Wqg, bqg, Wkg, bkg, Wvg, bvg,
                 attn_mask):
    """Build (nc, in_maps, meta) for the SPMD run."""
    hs = np.asarray(hidden_states, np.float32)
    m = np.asarray(attn_mask).reshape(-1).astype(np.int64)
    assert hs.shape == (1, SEQ, DIM) and m.shape == (SEQ,)
    is_pad, is_glob, globpos, ng, wtot, biasv, vidx, gbias, notpad = _mask_tables(m)
    nc = _get_program(ng, wtot, vidx, globpos, len(biasv))

    hsT = np.ascontiguousarray(hs[0].T, np.float32)
    ident = np.eye(P, dtype=np.float32)
    shared = {"hsT": hsT, "biasv": biasv, "gbias": gbias, "notpad": notpad, "ident": ident}
    args = [np.asarray(a, np.float32) for a in
            (Wq, bq, Wk, bk, Wv, bv, Wqg, bqg, Wkg, bkg, Wvg, bvg)]
    in_maps = []
    for h in range(N_CORES):
        wpack, bpack = _pack_weights(*args, h)
        in_maps.append(dict(shared, wpack=wpack, bpack=bpack))
    meta = (globpos, ng, wtot)
    return nc, in_maps, meta


def assemble(results, meta):
    """Scatter compact per-core outputs into full (out, attn)."""
    globpos, ng, wtot = meta
    attn = np.zeros((1, HEADS, SEQ, SEQ), np.float32)
    out = np.zeros((1, SEQ, DIM), np.float32)
    for h in range(N_CORES):
        r = results[h]
        ah = attn[0, h]
        ps = r["pstrips"]                       # (NT, P, wtot)
        for t in range(NT):
            lo, off, wid = _strip_geom(t)
            ah[t * P:(t + 1) * P, lo:lo + wid] = ps[t][:, off:off + wid]
        if ng:
            ah[:, globpos] = ps[:, :, 384:384 + ng].reshape(SEQ, ng)
            ah[globpos, :] = r["growattn"]
        out[0, :, h * HD:(h + 1) * HD] = r["ctx"]
        if ng:
            out[0, globpos, h * HD:(h + 1) * HD] = r["gctx"]
    return out, attn


def kernel(hidden_states, Wq, bq, Wk, bk, Wv, bv, Wqg, bqg, Wkg, bkg, Wvg, bvg,
           attn_mask):
    nc, in_maps, meta = make_in_maps(hidden_states, Wq, bq, Wk, bk, Wv, bv,
                                     Wqg, bqg, Wkg, bkg, Wvg, bvg, attn_mask)
    res = run_bass_kernel_spmd(nc, in_maps, core_ids=list(range(N_CORES)))
    return assemble(res.results, meta)


# revision 13
# speedup vs baseline: 1.6347x; 1.6347x over previous
"""Trainium2 Bass kernel for nn_LocalGlobalAttention (band + global-token attention).

Sharding: tensor-parallel over the 8 heads — one head per NeuronCore, no
collectives. Host concatenates per-head results.

Device computes, per head (all in key-major "plane" layout):
  planes[k]   = exp(K_k^T Q_win + band_bias)   (128 keys x <=384 queries)
  globplane   = exp(Kglob^T Q)                 (ng global keys x 2048 queries)
  ctxT        = V^T-weighted accumulation over planes       (64 x 2048)
  growraw     = exp(Qglob^T K) for the ng global query rows  (ng x 2048)
  pgst/gctxT  = the global-attention pass at the ng global query positions
Softmax normalization (row sums, 1/W scaling, pad masking) happens on the
host, which also scatters the compact planes into the dense attention output.
Masked entries are exact zeros (bias -1e4 underflows exp to 0.0 in fp32),
so the banded+global sparsity is exact, not approximate.

Matmuls run as float32r (TF32-like, ~1e-4) with fp32 PSUM accumulation.
"""
import os
import sys
from contextlib import ExitStack

for _p in ("/opt/trn_rl_repo", "/root/.axon_site/_ro/trn_rl_repo"):
    if os.path.isdir(_p) and _p not in sys.path:
        sys.path.insert(0, _p)

import numpy as np

import concourse.bass as bass
import concourse.tile as tile
from concourse import bacc, mybir
from concourse.bass_utils import run_bass_kernel_spmd

SEQ = 2048
DIM = 512
HEADS = 8
HD = 64          # head dim
WIN = 128        # local attention window
NEG = -10000.0
P = 128          # partitions
NT = SEQ // P    # 16 key/query tiles
KC = DIM // P    # 4 contraction chunks
NSC = SEQ // 512  # 4 sequence chunks of 512
SCALE = 1.0 / np.sqrt(HD)
F32 = mybir.dt.float32
F32R = mybir.dt.float32r

N_CORES = 8


def _plane_geom(k):
    """Plane k covers queries of tiles k-1..k+1 at window offsets 0/128/256."""
    tmin, tmax = max(k - 1, 0), min(k + 1, NT - 1)
    qlo, qhi = tmin * P, (tmax + 1) * P
    vlo = (1 - k + tmin) * P
    vhi = (1 - k + tmax) * P + P
    return tmin, tmax, qlo, qhi, vlo, vhi


def _mask_tables(m):
    """Host-side mask preprocessing. m: (SEQ,) int (1 real, -1 global, 0 pad)."""
    is_pad = m == 0
    is_glob = m == -1
    globpos = np.flatnonzero(is_glob).astype(np.int64)
    ng = len(globpos)

    a = np.arange(P)
    band_src = np.zeros((P, 3, P), np.float32)
    band_src[:, 0, :] = np.where(a[:, None] <= a[None, :], 0.0, NEG)
    band_src[:, 2, :] = np.where(a[:, None] >= a[None, :], 0.0, NEG)

    dead = is_pad | is_glob                      # keys excluded from band planes
    bias_pt = np.where(dead.reshape(NT, P).T, NEG, 0.0).astype(np.float32)   # (P, NT)
    bias_ptg = np.where(is_pad.reshape(NT, P).T, NEG, 0.0).astype(np.float32)
    gsec = (np.where(is_pad[globpos], NEG, 0.0).astype(np.float32).reshape(ng, 1)
            if ng else np.zeros((1, 1), np.float32))
    notpad = (~is_pad).astype(np.float32)        # (SEQ,)
    return is_pad, globpos, ng, band_src, bias_pt, bias_ptg, gsec, notpad


def _build_program(ng, globpos, biasptg_runs):
    """Trace the per-core Bass/Tile program (SPMD: same program on all cores)."""
    Exp = mybir.ActivationFunctionType.Exp
    nc = bacc.Bacc("TRN2", target_bir_lowering=False, debug=False, num_devices=1)

    hsT = nc.dram_tensor("hsT", (DIM, SEQ), F32, kind="ExternalInput").ap()
    wpack = nc.dram_tensor("wpack", (DIM, 384), F32, kind="ExternalInput").ap()
    bpack = nc.dram_tensor("bpack", (P, 3), F32, kind="ExternalInput").ap()
    band = nc.dram_tensor("band", (P, 3, P), F32, kind="ExternalInput").ap()
    biaspt = nc.dram_tensor("biaspt", (P, NT), F32, kind="ExternalInput").ap()
    biasptg = nc.dram_tensor("biasptg", (P, NT), F32, kind="ExternalInput").ap()
    gsec = nc.dram_tensor("gsec", (max(ng, 1), 1), F32, kind="ExternalInput").ap()
    ident = nc.dram_tensor("ident", (P, P), F32, kind="ExternalInput").ap()

    planes_out = nc.dram_tensor("planes", (P, NT, 384), F32, kind="ExternalOutput").ap()
    ctxT_out = nc.dram_tensor("ctxT", (HD, SEQ), F32, kind="ExternalOutput").ap()
    if ng:
        globplane_out = nc.dram_tensor("globplane", (ng, SEQ), F32, kind="ExternalOutput").ap()
        grow_out = nc.dram_tensor("growraw", (ng, SEQ), F32, kind="ExternalOutput").ap()
        pgst_out = nc.dram_tensor("pgst", (P, NT, ng), F32, kind="ExternalOutput").ap()
        gctxT_out = nc.dram_tensor("gctxT", (HD, ng), F32, kind="ExternalOutput").ap()

    uniform = False
    if ng >= 2:
        d = np.diff(globpos)
        uniform = len(set(d.tolist())) == 1
        g0, gst = int(globpos[0]), int(d[0])
    elif ng == 1:
        uniform, g0, gst = True, int(globpos[0]), 1

    with tile.TileContext(nc) as tc:
        with ExitStack() as ctx:
            consts = ctx.enter_context(tc.tile_pool(name="consts", bufs=1))

            # ---- load inputs; stage fp32, round to f32r on DVE ----
            staging = tc.tile_pool(name="staging", bufs=1)
            stg = staging.__enter__()
            hs_sb = stg.tile([P, KC, SEQ], F32)
            w_sb = stg.tile([P, KC, 384], F32)
            id_sb = stg.tile([P, P], F32)
            band_sb = stg.tile([P, 3, P], F32)
            for c in range(KC):
                nc.sync.dma_start(hs_sb[:, c], hsT[c * P:(c + 1) * P, :])
                nc.sync.dma_start(w_sb[:, c], wpack[c * P:(c + 1) * P, :])
            nc.sync.dma_start(id_sb[:], ident)
            nc.sync.dma_start(band_sb[:], band)
            bp_sb = consts.tile([P, 3], F32)
            nc.sync.dma_start(bp_sb[:], bpack)
            biaspt_sb = consts.tile([P, NT], F32)
            nc.sync.dma_start(biaspt_sb[:], biaspt)
            biasptg_sb = consts.tile([P, NT], F32)
            nc.sync.dma_start(biasptg_sb[:], biasptg)
            gsec_sb = consts.tile([max(ng, 1), 1], F32)
            nc.sync.dma_start(gsec_sb[:], gsec)

            hs_r = consts.tile([P, KC, SEQ], F32R)
            w_r = consts.tile([P, KC, 384], F32R)
            for c in range(KC):
                nc.vector.tensor_copy(hs_r[:, c], hs_sb[:, c])
                nc.vector.tensor_copy(w_r[:, c], w_sb[:, c])
            id_r = consts.tile([P, P], F32R)
            nc.vector.tensor_copy(id_r[:], id_sb[:])
            band_r = consts.tile([P, 3, P], F32R)
            nc.vector.tensor_copy(band_r[:], band_sb[:])
            zeros_r = consts.tile([P, HD], F32R)
            nc.vector.tensor_scalar(out=zeros_r[:], in0=hs_r[:, 0, 0:HD],
                                    scalar1=0.0, scalar2=None,
                                    op0=mybir.AluOpType.mult)
            staging.__exit__(None, None, None)

            # ---- projections: 3 packs of [2 x 64 features] over full seq ----
            # pack0 = [q|qg], pack1 = [k|kg], pack2 = [v|vg]; q/qg pre-scaled.
            proj = [consts.tile([P, SEQ], F32R, tag=f"proj{p}", name=f"proj{p}")
                    for p in range(3)]
            with tc.tile_pool(name="ps_proj", bufs=2, space="PSUM") as ps_proj, \
                 tc.tile_pool(name="ps_vb", bufs=2, space="PSUM") as ps_vb:
                for p in range(3):
                    for s in range(NSC):
                        psa = ps_proj.tile([P, 512], F32, tag="proj", name="psa")
                        for c in range(KC):
                            nc.tensor.matmul(psa[:], w_r[:, c, p * P:(p + 1) * P],
                                             hs_r[:, c, s * 512:(s + 1) * 512],
                                             start=(c == 0), stop=(c == KC - 1))
                        nc.vector.tensor_scalar(
                            out=proj[p][:, s * 512:(s + 1) * 512], in0=psa[:],
                            scalar1=bp_sb[:, p:p + 1], scalar2=None,
                            op0=mybir.AluOpType.add)

                qT = proj[0][0:64, :]
                qgT = proj[0][64:128, :]
                kT = proj[1][0:64, :]
                kgT = proj[1][64:128, :]
                vT = proj[2][0:64, :]
                vgT = proj[2][64:128, :]

                gcol_stages = []

                def glob_cols(src):
                    """(64, SEQ) AP -> (64, ng) AP of the global key columns."""
                    if uniform:
                        return src[:, g0:g0 + gst * ng:gst]
                    st = consts.tile([64, ng], F32R, tag=f"gcols{len(gcol_stages)}",
                                     name=f"gcols{len(gcol_stages)}")
                    gcol_stages.append(st)
                    for i, pp in enumerate(globpos):
                        nc.vector.tensor_copy(st[:, i:i + 1].bitcast(F32),
                                              src[:, int(pp):int(pp) + 1].bitcast(F32))
                    return st[:]

                # ---- V / Vg in (keys, 64) layout via PE transposes ----
                v_r = consts.tile([P, NT, HD], F32R)
                vg_r = consts.tile([P, NT, HD], F32R)
                for g in range(NT // 4):
                    vps = ps_vb.tile([P, 4 * HD], F32R, tag="vb", name="vps")
                    for i in range(4):
                        c = 4 * g + i
                        nc.tensor.transpose(vps[:, i * HD:(i + 1) * HD],
                                            vT[:, c * P:(c + 1) * P], id_r[0:64, 0:64])
                    nc.vector.tensor_copy(v_r[:, 4 * g:4 * g + 4, :], vps[:])
                if ng:
                    for g in range(NT // 4):
                        vps = ps_vb.tile([P, 4 * HD], F32R, tag="vb", name="vps")
                        for i in range(4):
                            c = 4 * g + i
                            nc.tensor.transpose(vps[:, i * HD:(i + 1) * HD],
                                                vgT[:, c * P:(c + 1) * P],
                                                id_r[64:128, 64:128])
                        nc.vector.tensor_copy(vg_r[:, 4 * g:4 * g + 4, :], vps[:])
                    vglob_r = consts.tile([ng, HD], F32R)
                    vps = ps_vb.tile([P, 4 * HD], F32R, tag="vb", name="vps")
                    nc.tensor.transpose(vps[0:ng, 0:HD], glob_cols(vT), id_r[0:64, 0:64])
                    nc.vector.tensor_copy(vglob_r[:], vps[0:ng, 0:HD])
                    kglob = glob_cols(kT)
                    qTg = glob_cols(qT)
                    qgTg = glob_cols(qgT)
                    if uniform:
                        kg_mm = kgT
                    else:
                        kg0 = consts.tile([64, SEQ], F32R)
                        nc.vector.tensor_copy(kg0[:].bitcast(F32), kgT.bitcast(F32))
                        kg_mm = kg0[:]

            # ---- score planes + ctxT ----
            planes = consts.tile([P, NT, 384], F32R)
            for k in range(NT):
                _, _, _, _, vlo, vhi = _plane_geom(k)
                if vlo > 0:
                    nc.vector.memset(planes[:, k, 0:vlo].bitcast(F32), 0.0)
                if vhi < 384:
                    nc.vector.memset(planes[:, k, vhi:384].bitcast(F32), 0.0)
            ctxT_sb = consts.tile([HD, SEQ], F32)
            with tc.tile_pool(name="ps_sm", bufs=3, space="PSUM") as ps_sm, \
                 tc.tile_pool(name="ps_ctxT", bufs=1, space="PSUM") as ps_ctxT:
                for k in range(NT):
                    tmin, tmax, qlo, qhi, vlo, vhi = _plane_geom(k)
                    spt = ps_sm.tile([P, 384], F32, tag="sm", name="spt")
                    for t in range(tmin, tmax + 1):
                        w = (1 - k + t) * P
                        nc.tensor.matmul(spt[:, w:w + P].bitcast(F32R),
                                         band_r[:, 1 + (k - t), :], id_r[:],
                                         is_transpose=True, start=(t == tmin),
                                         stop=False, skip_group_check=True)
                    nc.tensor.matmul(spt[:, vlo:vhi],
                                     kT[:, k * P:(k + 1) * P], qT[:, qlo:qhi],
                                     start=False, stop=True, skip_group_check=True)
                    nc.scalar.activation(planes[:, k, vlo:vhi], spt[:, vlo:vhi],
                                         Exp, bias=biaspt_sb[:, k:k + 1], scale=1.0)
                # raw-exp planes out (host does softmax normalization)
                pfl = planes[:].rearrange("p n w -> p (n w)")
                ofl = planes_out.rearrange("p n w -> p (n w)")
                for g in range(NT // 4):
                    nc.sync.dma_start(ofl[:, g * 4 * 384:(g + 1) * 4 * 384],
                                      pfl[:, g * 4 * 384:(g + 1) * 4 * 384].bitcast(F32))

                if ng:
                    globplane = consts.tile([ng, SEQ], F32R)
                    for s in range(NSC):
                        gpt = ps_sm.tile([P, 512], F32, tag="sm", name="gpt")
                        nc.tensor.matmul(gpt[0:ng, :], kglob,
                                         qT[:, s * 512:(s + 1) * 512],
                                         start=True, stop=True)
                        nc.scalar.activation(globplane[:, s * 512:(s + 1) * 512],
                                             gpt[0:ng, :], Exp, bias=gsec_sb[:],
                                             scale=1.0)
                    nc.sync.dma_start(globplane_out, globplane[:].bitcast(F32))

                # ctxT = V^T-weighted accumulation over planes (+ glob plane)
                ctps = ps_ctxT.tile([HD, SEQ], F32, tag="ctxT", name="ctps")
                for s in range(NSC):
                    nc.tensor.matmul(ctps[:, s * 512:(s + 1) * 512], zeros_r[:],
                                     hs_r[:, 0, s * 512:(s + 1) * 512],
                                     start=True, stop=False, skip_group_check=True)
                for k in range(NT):
                    tmin, tmax, qlo, qhi, vlo, vhi = _plane_geom(k)
                    # split at PSUM bank (512-col) boundaries
                    cuts = [qlo] + [b for b in (512, 1024, 1536) if qlo < b < qhi] + [qhi]
                    for a, b in zip(cuts, cuts[1:]):
                        nc.tensor.matmul(ctps[:, a:b], v_r[:, k, :],
                                         planes[:, k, vlo + (a - qlo):vlo + (b - qlo)],
                                         start=False, stop=False, skip_group_check=True)
                if ng:
                    for s in range(NSC):
                        nc.tensor.matmul(ctps[:, s * 512:(s + 1) * 512], vglob_r[:],
                                         globplane[:, s * 512:(s + 1) * 512],
                                         start=False, stop=True, skip_group_check=True)
                else:
                    for s in range(NSC):
                        nc.tensor.matmul(ctps[:, s * 512:(s + 1) * 512], zeros_r[:],
                                         hs_r[:, 0, s * 512:(s + 1) * 512],
                                         start=False, stop=True, skip_group_check=True)
                for s in range(NSC):
                    nc.scalar.copy(ctxT_sb[:, s * 512:(s + 1) * 512],
                                   ctps[:, s * 512:(s + 1) * 512])
                nc.sync.dma_start(ctxT_out, ctxT_sb[:])

                # ---- dense passes at the ng global positions ----
                if ng:
                    # local-attention rows of the global queries (for attn output)
                    growraw = consts.tile([ng, SEQ], F32R)
                    for s in range(NSC):
                        grp = ps_sm.tile([P, 512], F32, tag="sm", name="grp")
                        nc.tensor.matmul(grp[0:ng, :], qTg,
                                         kT[:, s * 512:(s + 1) * 512],
                                         start=True, stop=True)
                        nc.scalar.activation(growraw[:, s * 512:(s + 1) * 512],
                                             grp[0:ng, :], Exp, bias=gsec_sb[:],
                                             scale=1.0)
                    nc.sync.dma_start(grow_out, growraw[:].bitcast(F32))

                    # global-attention pass: pg planes + gctxT
                    pgst = consts.tile([P, NT, ng], F32R)
                    for g in range(NT // 4):
                        pgp = ps_sm.tile([P, 4 * ng], F32, tag="sm", name="pgp")
                        for i in range(4):
                            c = 4 * g + i
                            nc.tensor.matmul(pgp[:, i * ng:(i + 1) * ng],
                                             kg_mm[:, c * P:(c + 1) * P], qgTg,
                                             start=True, stop=True)
                        for (i0, ilen) in biasptg_runs[g]:
                            nc.scalar.activation(
                                pgst[:, 4 * g + i0:4 * g + i0 + ilen, :],
                                pgp[:, i0 * ng:(i0 + ilen) * ng], Exp,
                                bias=biasptg_sb[:, 4 * g + i0:4 * g + i0 + 1],
                                scale=1.0)
                    nc.sync.dma_start(
                        pgst_out.rearrange("p n w -> p (n w)"),
                        pgst[:].rearrange("p n w -> p (n w)").bitcast(F32))
                    gcps = ps_sm.tile([HD, ng], F32, tag="sm", name="gcps")
                    for k in range(NT):
                        nc.tensor.matmul(gcps[:], vg_r[:, k, :], pgst[:, k, :],
                                         start=(k == 0), stop=(k == NT - 1))
                    gctxT_sb = consts.tile([HD, ng], F32)
                    nc.scalar.copy(gctxT_sb[:], gcps[:])
                    nc.sync.dma_start(gctxT_out, gctxT_sb[:])

    nc.compile()
    return nc


_PROGRAM_CACHE = {}


def _biasptg_runs(bias_ptg):
    """Per psum-group-of-4: runs of consecutive key tiles with identical
    pad-mask bias columns (each run shares one exp instruction)."""
    runs = []
    for g in range(NT // 4):
        r, i = [], 0
        while i < 4:
            j = i
            while (j + 1 < 4 and np.array_equal(bias_ptg[:, 4 * g + j + 1],
                                                bias_ptg[:, 4 * g + i])):
                j += 1
            r.append((i, j - i + 1))
            i = j + 1
        runs.append(r)
    return runs


def _get_program(ng, globpos, bias_ptg):
    runs = _biasptg_runs(bias_ptg)
    key = (ng, tuple(globpos.tolist()), bias_ptg.tobytes())
    if key not in _PROGRAM_CACHE:
        _PROGRAM_CACHE[key] = _build_program(ng, globpos, runs)
    return _PROGRAM_CACHE[key]


def _pack_weights(Wq, bq, Wk, bk, Wv, bv, Wqg, bqg, Wkg, bkg, Wvg, bvg, h):
    """Per-head packed weight (DIM, 384) and bias (P, 3), scale folded into q/qg."""
    sl = slice(h * HD, (h + 1) * HD)
    wpack = np.concatenate([Wq[sl].T * SCALE, Wqg[sl].T * SCALE,
                            Wk[sl].T, Wkg[sl].T,
                            Wv[sl].T, Wvg[sl].T], axis=1)
    bpack = np.stack([np.concatenate([bq[sl] * SCALE, bqg[sl] * SCALE]),
                      np.concatenate([bk[sl], bkg[sl]]),
                      np.concatenate([bv[sl], bvg[sl]])], axis=1)
    return np.ascontiguousarray(wpack, np.float32), np.ascontiguousarray(bpack, np.float32)


def make_in_maps(hidden_states, Wq, bq, Wk, bk, Wv, bv, Wqg, bqg, Wkg, bkg, Wvg, bvg,
                 attn_mask):
    """Build (nc, in_maps, meta) for the SPMD run."""
    hs = np.asarray(hidden_states, np.float32)
    m = np.asarray(attn_mask).reshape(-1).astype(np.int64)
    assert hs.shape == (1, SEQ, DIM) and m.shape == (SEQ,)
    is_pad, globpos, ng, band_src, bias_pt, bias_ptg, gsec, notpad = _mask_tables(m)
    nc = _get_program(ng, globpos, bias_ptg)

    hsT = np.ascontiguousarray(hs[0].T, np.float32)
    ident = np.eye(P, dtype=np.float32)
    shared = {"hsT": hsT, "band": band_src, "biaspt": bias_pt, "biasptg": bias_ptg,
              "gsec": gsec, "ident": ident}
    args = [np.asarray(a, np.float32) for a in
            (Wq, bq, Wk, bk, Wv, bv, Wqg, bqg, Wkg, bkg, Wvg, bvg)]
    in_maps = []
    for h in range(N_CORES):
        wpack, bpack = _pack_weights(*args, h)
        in_maps.append(dict(shared, wpack=wpack, bpack=bpack))
    meta = (globpos, ng, notpad)
    return nc, in_maps, meta


def assemble(results, meta):
    """Host: scatter planes into dense attn, normalize softmax, build out."""
    globpos, ng, notpad = meta
    attn = np.zeros((1, HEADS, SEQ, SEQ), np.float32)
    out = np.zeros((1, SEQ, DIM), np.float32)
    for h in range(N_CORES):
        r = results[h]
        A = attn[0, h]
        pl = r["planes"]                                    # (P, NT, 384)
        for k in range(NT):
            tmin, tmax, _, _, _, _ = _plane_geom(k)
            for t in range(tmin, tmax + 1):
                w = (1 - k + t) * P
                A[t * P:(t + 1) * P, k * P:(k + 1) * P] = pl[:, k, w:w + P].T
        if ng:
            A[:, globpos] = r["globplane"].T
        W = A.sum(axis=1)
        W[W == 0] = 1.0
        scl = notpad / W
        A *= scl[:, None]
        if ng:
            grow = r["growraw"] * notpad[None, :]
            gw = grow.sum(axis=1, keepdims=True)
            gw[gw == 0] = 1.0
            A[globpos, :] = grow / gw
        out[0, :, h * HD:(h + 1) * HD] = r["ctxT"].T * scl[:, None]
        if ng:
            wg = r["pgst"].sum(axis=(0, 1))                 # (ng,)
            wg[wg == 0] = 1.0
            out[0, globpos, h * HD:(h + 1) * HD] = r["gctxT"].T / wg[:, None]
    return out, attn


def kernel(hidden_states, Wq, bq, Wk, bk, Wv, bv, Wqg, bqg, Wkg, bkg, Wvg, bvg,
           attn_mask):
    nc, in_maps, meta = make_in_maps(hidden_states, Wq, bq, Wk, bk, Wv, bv,
                                     Wqg, bqg, Wkg, bkg, Wvg, bvg, attn_mask)
    res = run_bass_kernel_spmd(nc, in_maps, core_ids=list(range(N_CORES)))
    return assemble(res.results, meta)


# revision 15
# speedup vs baseline: 1.9458x; 1.1903x over previous
"""Trainium2 Bass kernel for nn_LocalGlobalAttention (band + global-token attention).

Sharding: tensor-parallel over the 8 heads — one head per NeuronCore, no
collectives. Host concatenates per-head results.

Device computes, per head (all in key-major "plane" layout):
  planes[k]   = exp(K_k^T Q_win + band_bias)   (128 keys x <=384 queries)
  globplane   = exp(Kglob^T Q)                 (ng global keys x 2048 queries)
  ctxT        = V^T-weighted accumulation over planes       (64 x 2048)
  growraw     = exp(Qglob^T K) for the ng global query rows  (ng x 2048)
  pgst/gctxT  = the global-attention pass at the ng global query positions
Softmax normalization (row sums, 1/W scaling, pad masking) happens on the
host, which also scatters the compact planes into the dense attention output.
Masked entries are exact zeros (bias -1e4 underflows exp to 0.0 in fp32),
so the banded+global sparsity is exact, not approximate.

Matmuls run as float32r (TF32-like, ~1e-4) with fp32 PSUM accumulation.
"""
import os
import sys
from contextlib import ExitStack

for _p in ("/opt/trn_rl_repo", "/root/.axon_site/_ro/trn_rl_repo"):
    if os.path.isdir(_p) and _p not in sys.path:
        sys.path.insert(0, _p)

import numpy as np

import concourse.bass as bass
import concourse.tile as tile
from concourse import bacc, mybir
from concourse.bass_utils import run_bass_kernel_spmd

SEQ = 2048
DIM = 512
HEADS = 8
HD = 64          # head dim
WIN = 128        # local attention window
NEG = -10000.0
P = 128          # partitions
NT = SEQ // P    # 16 key/query tiles
KC = DIM // P    # 4 contraction chunks
NSC = SEQ // 512  # 4 sequence chunks of 512
SCALE = 1.0 / np.sqrt(HD)
F32 = mybir.dt.float32
F32R = mybir.dt.float32r
F16 = mybir.dt.float16

N_CORES = 8


def _plane_geom(k):
    """Plane k covers queries of tiles k-1..k+1 at window offsets 0/128/256."""
    tmin, tmax = max(k - 1, 0), min(k + 1, NT - 1)
    qlo, qhi = tmin * P, (tmax + 1) * P
    vlo = (1 - k + tmin) * P
    vhi = (1 - k + tmax) * P + P
    return tmin, tmax, qlo, qhi, vlo, vhi


def _mask_tables(m):
    """Host-side mask preprocessing. m: (SEQ,) int (1 real, -1 global, 0 pad)."""
    is_pad = m == 0
    is_glob = m == -1
    globpos = np.flatnonzero(is_glob).astype(np.int64)
    ng = len(globpos)

    a = np.arange(P)
    band_src = np.zeros((P, 3, P), np.float32)
    band_src[:, 0, :] = np.where(a[:, None] <= a[None, :], 0.0, NEG)
    band_src[:, 2, :] = np.where(a[:, None] >= a[None, :], 0.0, NEG)

    dead = is_pad | is_glob                      # keys excluded from band planes
    bias_pt = np.where(dead.reshape(NT, P).T, NEG, 0.0).astype(np.float32)   # (P, NT)
    bias_ptg = np.where(is_pad.reshape(NT, P).T, NEG, 0.0).astype(np.float32)
    gsec = (np.where(is_pad[globpos], NEG, 0.0).astype(np.float32).reshape(ng, 1)
            if ng else np.zeros((1, 1), np.float32))
    notpad = (~is_pad).astype(np.float32)        # (SEQ,)
    return is_pad, globpos, ng, band_src, bias_pt, bias_ptg, gsec, notpad


def _build_program(ng, globpos, biasptg_runs):
    """Trace the per-core Bass/Tile program (SPMD: same program on all cores)."""
    Exp = mybir.ActivationFunctionType.Exp
    nc = bacc.Bacc("TRN2", target_bir_lowering=False, debug=False, num_devices=1)

    hsT = nc.dram_tensor("hsT", (DIM, SEQ), F16, kind="ExternalInput").ap()
    wpack = nc.dram_tensor("wpack", (DIM, 384), F16, kind="ExternalInput").ap()
    bpack = nc.dram_tensor("bpack", (P, 3), F32, kind="ExternalInput").ap()
    band = nc.dram_tensor("band", (P, 3, P), F32, kind="ExternalInput").ap()
    biaspt = nc.dram_tensor("biaspt", (P, NT), F32, kind="ExternalInput").ap()
    biasptg = nc.dram_tensor("biasptg", (P, NT), F32, kind="ExternalInput").ap()
    gsec = nc.dram_tensor("gsec", (max(ng, 1), 1), F32, kind="ExternalInput").ap()
    ident = nc.dram_tensor("ident", (P, P), F32, kind="ExternalInput").ap()

    planes_out = nc.dram_tensor("planes", (P, NT, 384), F32, kind="ExternalOutput").ap()
    ctxT_out = nc.dram_tensor("ctxT", (HD, SEQ), F32, kind="ExternalOutput").ap()
    if ng:
        globplane_out = nc.dram_tensor("globplane", (ng, SEQ), F32, kind="ExternalOutput").ap()
        grow_out = nc.dram_tensor("growraw", (ng, SEQ), F32, kind="ExternalOutput").ap()
        pgst_out = nc.dram_tensor("pgst", (P, NT, ng), F32, kind="ExternalOutput").ap()
        gctxT_out = nc.dram_tensor("gctxT", (HD, ng), F32, kind="ExternalOutput").ap()

    uniform = False
    if ng >= 2:
        d = np.diff(globpos)
        uniform = len(set(d.tolist())) == 1
        g0, gst = int(globpos[0]), int(d[0])
    elif ng == 1:
        uniform, g0, gst = True, int(globpos[0]), 1

    with tile.TileContext(nc) as tc:
        with ExitStack() as ctx:
            consts = ctx.enter_context(tc.tile_pool(name="consts", bufs=1))

            # ---- load inputs; stage fp32, round to f32r on DVE ----
            staging = tc.tile_pool(name="staging", bufs=1)
            stg = staging.__enter__()
            id_sb = stg.tile([P, P], F32)
            band_sb = stg.tile([P, 3, P], F32)
            hs_h = consts.tile([P, KC, SEQ], F16)
            w_h = consts.tile([P, KC, 384], F16)
            for c in range(KC):
                nc.sync.dma_start(w_h[:, c], wpack[c * P:(c + 1) * P, :])
                nc.sync.dma_start(hs_h[:, c], hsT[c * P:(c + 1) * P, :])
            nc.sync.dma_start(id_sb[:], ident)
            nc.sync.dma_start(band_sb[:], band)
            bp_sb = consts.tile([P, 3], F32)
            nc.sync.dma_start(bp_sb[:], bpack)
            biaspt_sb = consts.tile([P, NT], F32)
            nc.sync.dma_start(biaspt_sb[:], biaspt)
            biasptg_sb = consts.tile([P, NT], F32)
            nc.sync.dma_start(biasptg_sb[:], biasptg)
            gsec_sb = consts.tile([max(ng, 1), 1], F32)
            nc.sync.dma_start(gsec_sb[:], gsec)

            id_r = consts.tile([P, P], F32R)
            nc.vector.tensor_copy(id_r[:], id_sb[:])
            band_r = consts.tile([P, 3, P], F32R)
            nc.vector.tensor_copy(band_r[:], band_sb[:])
            zeros_r = consts.tile([P, HD], F32R)
            nc.vector.tensor_scalar(out=zeros_r[:], in0=id_sb[:, 0:HD],
                                    scalar1=0.0, scalar2=None,
                                    op0=mybir.AluOpType.mult)
            staging.__exit__(None, None, None)

            # ---- projections + attention, phase-ordered for DMA overlap ----
            # pack0 = [q|qg], pack1 = [k|kg], pack2 = [v|vg]; q/qg pre-scaled.
            Exp = mybir.ActivationFunctionType.Exp
            proj = [consts.tile([P, SEQ], F32R, tag=f"proj{p}", name=f"proj{p}")
                    for p in range(3)]
            ps = ctx.enter_context(tc.tile_pool(name="ps", bufs=3, space="PSUM"))

            def do_pack(p):
                for s in range(NSC):
                    psa = ps.tile([P, 512], F32, tag="sm", name="psa")
                    for c in range(KC):
                        nc.tensor.matmul(psa[:], w_h[:, c, p * P:(p + 1) * P],
                                         hs_h[:, c, s * 512:(s + 1) * 512],
                                         start=(c == 0), stop=(c == KC - 1))
                    nc.vector.tensor_scalar(
                        out=proj[p][:, s * 512:(s + 1) * 512], in0=psa[:],
                        scalar1=bp_sb[:, p:p + 1], scalar2=None,
                        op0=mybir.AluOpType.add)

            do_pack(0)
            do_pack(1)
            qT = proj[0][0:64, :]
            qgT = proj[0][64:128, :]
            kT = proj[1][0:64, :]
            kgT = proj[1][64:128, :]

            gcol_stages = []

            def glob_cols(src):
                """(64, SEQ) AP -> (64, ng) AP of the global key columns."""
                if uniform:
                    return src[:, g0:g0 + gst * ng:gst]
                st = consts.tile([64, ng], F32R, tag=f"gcols{len(gcol_stages)}",
                                 name=f"gcols{len(gcol_stages)}")
                gcol_stages.append(st)
                for i, pp in enumerate(globpos):
                    nc.vector.tensor_copy(st[:, i:i + 1].bitcast(F32),
                                          src[:, int(pp):int(pp) + 1].bitcast(F32))
                return st[:]

            # ---- early dense passes at the ng global positions (overlap DMA) ----
            if ng:
                kglob = glob_cols(kT)
                qTg = glob_cols(qT)
                qgTg = glob_cols(qgT)
                if uniform:
                    kg_mm = kgT
                else:
                    kg0 = consts.tile([64, SEQ], F32R)
                    nc.vector.tensor_copy(kg0[:].bitcast(F32), kgT.bitcast(F32))
                    kg_mm = kg0[:]

                globplane = consts.tile([ng, SEQ], F32R)
                for s in range(NSC):
                    gpt = ps.tile([P, 512], F32, tag="sm", name="gpt")
                    nc.tensor.matmul(gpt[0:ng, :], kglob,
                                     qT[:, s * 512:(s + 1) * 512],
                                     start=True, stop=True)
                    nc.scalar.activation(globplane[:, s * 512:(s + 1) * 512],
                                         gpt[0:ng, :], Exp, bias=gsec_sb[:],
                                         scale=1.0)
                nc.sync.dma_start(globplane_out, globplane[:].bitcast(F32))

                growraw = consts.tile([ng, SEQ], F32R)
                for s in range(NSC):
                    grp = ps.tile([P, 512], F32, tag="sm", name="grp")
                    nc.tensor.matmul(grp[0:ng, :], qTg,
                                     kT[:, s * 512:(s + 1) * 512],
                                     start=True, stop=True)
                    nc.scalar.activation(growraw[:, s * 512:(s + 1) * 512],
                                         grp[0:ng, :], Exp, bias=gsec_sb[:],
                                         scale=1.0)
                nc.sync.dma_start(grow_out, growraw[:].bitcast(F32))

                pgst = consts.tile([P, NT, ng], F32R)
                for g in range(NT // 4):
                    pgp = ps.tile([P, 4 * ng], F32, tag="sm", name="pgp")
                    for i in range(4):
                        c = 4 * g + i
                        nc.tensor.matmul(pgp[:, i * ng:(i + 1) * ng],
                                         kg_mm[:, c * P:(c + 1) * P], qgTg,
                                         start=True, stop=True)
                    for (i0, ilen) in biasptg_runs[g]:
                        nc.scalar.activation(
                            pgst[:, 4 * g + i0:4 * g + i0 + ilen, :],
                            pgp[:, i0 * ng:(i0 + ilen) * ng], Exp,
                            bias=biasptg_sb[:, 4 * g + i0:4 * g + i0 + 1],
                            scale=1.0)
                nc.sync.dma_start(
                    pgst_out.rearrange("p n w -> p (n w)"),
                    pgst[:].rearrange("p n w -> p (n w)").bitcast(F32))

            # ---- band score planes ----
            planes = consts.tile([P, NT, 384], F32R)
            for k in range(NT):
                _, _, _, _, vlo, vhi = _plane_geom(k)
                if vlo > 0:
                    nc.vector.memset(planes[:, k, 0:vlo].bitcast(F32), 0.0)
                if vhi < 384:
                    nc.vector.memset(planes[:, k, vhi:384].bitcast(F32), 0.0)
            pfl = planes[:].rearrange("p n w -> p (n w)")
            ofl = planes_out.rearrange("p n w -> p (n w)")
            for k in range(NT):
                tmin, tmax, qlo, qhi, vlo, vhi = _plane_geom(k)
                spt = ps.tile([P, 384], F32, tag="sm", name="spt")
                for t in range(tmin, tmax + 1):
                    w = (1 - k + t) * P
                    nc.tensor.matmul(spt[:, w:w + P].bitcast(F32R),
                                     band_r[:, 1 + (k - t), :], id_r[:],
                                     is_transpose=True, start=(t == tmin),
                                     stop=False, skip_group_check=True)
                nc.tensor.matmul(spt[:, vlo:vhi],
                                 kT[:, k * P:(k + 1) * P], qT[:, qlo:qhi],
                                 start=False, stop=True, skip_group_check=True)
                nc.scalar.activation(planes[:, k, vlo:vhi], spt[:, vlo:vhi],
                                     Exp, bias=biaspt_sb[:, k:k + 1], scale=1.0)
                if k % 2 == 1:
                    nc.sync.dma_start(ofl[:, (k - 1) * 384:(k + 1) * 384],
                                      pfl[:, (k - 1) * 384:(k + 1) * 384].bitcast(F32))

            # ---- pack2 (v|vg) + V/Vg in (keys, 64) layout ----
            do_pack(2)
            vT = proj[2][0:64, :]
            vgT = proj[2][64:128, :]
            v_r = consts.tile([P, NT, HD], F32R)
            vg_r = consts.tile([P, NT, HD], F32R)
            for g in range(NT // 4):
                vps = ps.tile([P, 4 * HD], F32R, tag="sm", name="vps")
                for i in range(4):
                    c = 4 * g + i
                    nc.tensor.transpose(vps[:, i * HD:(i + 1) * HD],
                                        vT[:, c * P:(c + 1) * P], id_r[0:64, 0:64])
                nc.vector.tensor_copy(v_r[:, 4 * g:4 * g + 4, :], vps[:])
            if ng:
                for g in range(NT // 4):
                    vps = ps.tile([P, 4 * HD], F32R, tag="sm", name="vps")
                    for i in range(4):
                        c = 4 * g + i
                        nc.tensor.transpose(vps[:, i * HD:(i + 1) * HD],
                                            vgT[:, c * P:(c + 1) * P],
                                            id_r[64:128, 64:128])
                    nc.vector.tensor_copy(vg_r[:, 4 * g:4 * g + 4, :], vps[:])
                vglob_r = consts.tile([ng, HD], F32R)
                vps = ps.tile([P, 4 * HD], F32R, tag="sm", name="vps")
                nc.tensor.transpose(vps[0:ng, 0:HD], glob_cols(vT), id_r[0:64, 0:64])
                nc.vector.tensor_copy(vglob_r[:], vps[0:ng, 0:HD])

            # ---- ctxT accumulation ----
            ctxT_sb = consts.tile([HD, SEQ], F32)
            with tc.tile_pool(name="ps_ctxT", bufs=1, space="PSUM") as ps_ctxT:
                ctps = ps_ctxT.tile([HD, SEQ], F32, tag="ctxT", name="ctps")
                for s in range(NSC):
                    nc.tensor.matmul(ctps[:, s * 512:(s + 1) * 512], zeros_r[:],
                                     proj[0][:, s * 512:(s + 1) * 512],
                                     start=True, stop=False, skip_group_check=True)
                for k in range(NT):
                    tmin, tmax, qlo, qhi, vlo, vhi = _plane_geom(k)
                    cuts = [qlo] + [b for b in (512, 1024, 1536) if qlo < b < qhi] + [qhi]
                    for a, b in zip(cuts, cuts[1:]):
                        nc.tensor.matmul(ctps[:, a:b], v_r[:, k, :],
                                         planes[:, k, vlo + (a - qlo):vlo + (b - qlo)],
                                         start=False, stop=False, skip_group_check=True)
                if ng:
                    for s in range(NSC):
                        nc.tensor.matmul(ctps[:, s * 512:(s + 1) * 512], vglob_r[:],
                                         globplane[:, s * 512:(s + 1) * 512],
                                         start=False, stop=True, skip_group_check=True)
                else:
                    for s in range(NSC):
                        nc.tensor.matmul(ctps[:, s * 512:(s + 1) * 512], zeros_r[:],
                                         proj[0][:, s * 512:(s + 1) * 512],
                                         start=False, stop=True, skip_group_check=True)
                for s in range(NSC):
                    nc.scalar.copy(ctxT_sb[:, s * 512:(s + 1) * 512],
                                   ctps[:, s * 512:(s + 1) * 512])
                nc.sync.dma_start(ctxT_out, ctxT_sb[:])

                if ng:
                    gcps = ps.tile([HD, ng], F32, tag="sm", name="gcps")
                    for k in range(NT):
                        nc.tensor.matmul(gcps[:], vg_r[:, k, :], pgst[:, k, :],
                                         start=(k == 0), stop=(k == NT - 1))
                    gctxT_sb = consts.tile([HD, ng], F32)
                    nc.scalar.copy(gctxT_sb[:], gcps[:])
                    nc.sync.dma_start(gctxT_out, gctxT_sb[:])

    nc.compile()
    return nc


_PROGRAM_CACHE = {}


def _biasptg_runs(bias_ptg):
    """Per psum-group-of-4: runs of consecutive key tiles with identical
    pad-mask bias columns (each run shares one exp instruction)."""
    runs = []
    for g in range(NT // 4):
        r, i = [], 0
        while i < 4:
            j = i
            while (j + 1 < 4 and np.array_equal(bias_ptg[:, 4 * g + j + 1],
                                                bias_ptg[:, 4 * g + i])):
                j += 1
            r.append((i, j - i + 1))
            i = j + 1
        runs.append(r)
    return runs


def _get_program(ng, globpos, bias_ptg):
    runs = _biasptg_runs(bias_ptg)
    key = (ng, tuple(globpos.tolist()), bias_ptg.tobytes())
    if key not in _PROGRAM_CACHE:
        _PROGRAM_CACHE[key] = _build_program(ng, globpos, runs)
    return _PROGRAM_CACHE[key]


def _pack_weights(Wq, bq, Wk, bk, Wv, bv, Wqg, bqg, Wkg, bkg, Wvg, bvg, h):
    """Per-head packed weight (DIM, 384) and bias (P, 3), scale folded into q/qg."""
    sl = slice(h * HD, (h + 1) * HD)
    wpack = np.concatenate([Wq[sl].T * SCALE, Wqg[sl].T * SCALE,
                            Wk[sl].T, Wkg[sl].T,
                            Wv[sl].T, Wvg[sl].T], axis=1).astype(np.float16)
    bpack = np.stack([np.concatenate([bq[sl] * SCALE, bqg[sl] * SCALE]),
                      np.concatenate([bk[sl], bkg[sl]]),
                      np.concatenate([bv[sl], bvg[sl]])], axis=1)
    return np.ascontiguousarray(wpack), np.ascontiguousarray(bpack, np.float32)


def make_in_maps(hidden_states, Wq, bq, Wk, bk, Wv, bv, Wqg, bqg, Wkg, bkg, Wvg, bvg,
                 attn_mask):
    """Build (nc, in_maps, meta) for the SPMD run."""
    hs = np.asarray(hidden_states, np.float32)
    m = np.asarray(attn_mask).reshape(-1).astype(np.int64)
    assert hs.shape == (1, SEQ, DIM) and m.shape == (SEQ,)
    is_pad, globpos, ng, band_src, bias_pt, bias_ptg, gsec, notpad = _mask_tables(m)
    nc = _get_program(ng, globpos, bias_ptg)

    hsT = np.ascontiguousarray(hs[0].T.astype(np.float16))
    ident = np.eye(P, dtype=np.float32)
    shared = {"hsT": hsT, "band": band_src, "biaspt": bias_pt, "biasptg": bias_ptg,
              "gsec": gsec, "ident": ident}
    args = [np.asarray(a, np.float32) for a in
            (Wq, bq, Wk, bk, Wv, bv, Wqg, bqg, Wkg, bkg, Wvg, bvg)]
    in_maps = []
    for h in range(N_CORES):
        wpack, bpack = _pack_weights(*args, h)
        in_maps.append(dict(shared, wpack=wpack, bpack=bpack))
    meta = (globpos, ng, notpad)
    return nc, in_maps, meta


def assemble(results, meta):
    """Host: scatter planes into dense attn, normalize softmax, build out."""
    globpos, ng, notpad = meta
    attn = np.zeros((1, HEADS, SEQ, SEQ), np.float32)
    out = np.zeros((1, SEQ, DIM), np.float32)
    for h in range(N_CORES):
        r = results[h]
        A = attn[0, h]
        pl = r["planes"]                                    # (P, NT, 384)
        for k in range(NT):
            tmin, tmax, _, _, _, _ = _plane_geom(k)
            for t in range(tmin, tmax + 1):
                w = (1 - k + t) * P
                A[t * P:(t + 1) * P, k * P:(k + 1) * P] = pl[:, k, w:w + P].T
        if ng:
            A[:, globpos] = r["globplane"].T
        W = A.sum(axis=1)
        W[W == 0] = 1.0
        scl = notpad / W
        A *= scl[:, None]
        if ng:
            grow = r["growraw"] * notpad[None, :]
            gw = grow.sum(axis=1, keepdims=True)
            gw[gw == 0] = 1.0
            A[globpos, :] = grow / gw
        out[0, :, h * HD:(h + 1) * HD] = r["ctxT"].T * scl[:, None]
        if ng:
            wg = r["pgst"].sum(axis=(0, 1))                 # (ng,)
            wg[wg == 0] = 1.0
            out[0, globpos, h * HD:(h + 1) * HD] = r["gctxT"].T / wg[:, None]
    return out, attn


def kernel(hidden_states, Wq, bq, Wk, bk, Wv, bv, Wqg, bqg, Wkg, bkg, Wvg, bvg,
           attn_mask):
    nc, in_maps, meta = make_in_maps(hidden_states, Wq, bq, Wk, bk, Wv, bv,
                                     Wqg, bqg, Wkg, bkg, Wvg, bvg, attn_mask)
    res = run_bass_kernel_spmd(nc, in_maps, core_ids=list(range(N_CORES)))
    return assemble(res.results, meta)


# revision 20
# speedup vs baseline: 2.2527x; 1.1577x over previous
"""Trainium2 Bass kernel for nn_LocalGlobalAttention (band + global-token attention).

Sharding: tensor-parallel over the 8 heads — one head per NeuronCore, no
collectives. Host concatenates per-head results.

Device computes, per head (all in key-major "plane" layout):
  planes[k]   = exp(K_k^T Q_win + band_bias)   (128 keys x <=384 queries)
  globplane   = exp(Kglob^T Q)                 (ng global keys x 2048 queries)
  ctxT        = V^T-weighted accumulation over planes       (64 x 2048)
  growraw     = exp(Qglob^T K) for the ng global query rows  (ng x 2048)
  pgst/gctxT  = the global-attention pass at the ng global query positions
Softmax normalization (row sums, 1/W scaling, pad masking) happens on the
host, which also scatters the compact planes into the dense attention output.
Masked entries are exact zeros (bias -1e4 underflows exp to 0.0 in fp32),
so the banded+global sparsity is exact, not approximate.

Matmuls run as float32r (TF32-like, ~1e-4) with fp32 PSUM accumulation.
"""
import os
import sys
from contextlib import ExitStack

for _p in ("/opt/trn_rl_repo", "/root/.axon_site/_ro/trn_rl_repo"):
    if os.path.isdir(_p) and _p not in sys.path:
        sys.path.insert(0, _p)

import numpy as np

import concourse.bass as bass
import concourse.tile as tile
from concourse import bacc, mybir
from concourse.bass_utils import run_bass_kernel_spmd

SEQ = 2048
DIM = 512
HEADS = 8
HD = 64          # head dim
WIN = 128        # local attention window
NEG = -10000.0
P = 128          # partitions
NT = SEQ // P    # 16 key/query tiles
KC = DIM // P    # 4 contraction chunks
NSC = SEQ // 512  # 4 sequence chunks of 512
SCALE = 1.0 / np.sqrt(HD)
F32 = mybir.dt.float32
F32R = mybir.dt.float32r
F16 = mybir.dt.float16

N_CORES = 8


def _plane_geom(k):
    """Plane k covers queries of tiles k-1..k+1 at window offsets 0/128/256."""
    tmin, tmax = max(k - 1, 0), min(k + 1, NT - 1)
    qlo, qhi = tmin * P, (tmax + 1) * P
    vlo = (1 - k + tmin) * P
    vhi = (1 - k + tmax) * P + P
    return tmin, tmax, qlo, qhi, vlo, vhi


def _mask_tables(m):
    """Host-side mask preprocessing. m: (SEQ,) int (1 real, -1 global, 0 pad)."""
    is_pad = m == 0
    is_glob = m == -1
    globpos = np.flatnonzero(is_glob).astype(np.int64)
    ng = len(globpos)

    a = np.arange(P)
    band_src = np.zeros((P, 3, P), np.float32)
    band_src[:, 0, :] = np.where(a[:, None] <= a[None, :], 0.0, NEG)
    band_src[:, 2, :] = np.where(a[:, None] >= a[None, :], 0.0, NEG)

    dead = is_pad | is_glob                      # keys excluded from band planes
    bias_pt = np.where(dead.reshape(NT, P).T, NEG, 0.0).astype(np.float32)   # (P, NT)
    bias_ptg = np.where(is_pad.reshape(NT, P).T, NEG, 0.0).astype(np.float32)
    gsec = (np.where(is_pad[globpos], NEG, 0.0).astype(np.float32).reshape(ng, 1)
            if ng else np.zeros((1, 1), np.float32))
    notpad = (~is_pad).astype(np.float32)        # (SEQ,)
    return is_pad, globpos, ng, band_src, bias_pt, bias_ptg, gsec, notpad


def _build_program(ng, globpos, biasptg_runs):
    """Trace the per-core Bass/Tile program (SPMD: same program on all cores)."""
    Exp = mybir.ActivationFunctionType.Exp
    nc = bacc.Bacc("TRN2", target_bir_lowering=False, debug=False, num_devices=1)

    hsT = nc.dram_tensor("hsT", (DIM, SEQ), F16, kind="ExternalInput").ap()
    wpack = nc.dram_tensor("wpack", (DIM, 384), F16, kind="ExternalInput").ap()
    bpack = nc.dram_tensor("bpack", (P, 3), F32, kind="ExternalInput").ap()
    band = nc.dram_tensor("band", (P, 3, P), F32, kind="ExternalInput").ap()
    biaspt = nc.dram_tensor("biaspt", (P, NT), F32, kind="ExternalInput").ap()
    biasptg = nc.dram_tensor("biasptg", (P, NT), F32, kind="ExternalInput").ap()
    gsec = nc.dram_tensor("gsec", (max(ng, 1), 1), F32, kind="ExternalInput").ap()
    ident = nc.dram_tensor("ident", (P, P), F32, kind="ExternalInput").ap()

    planes_out = nc.dram_tensor("planes", (P, NT, 384), F32, kind="ExternalOutput").ap()
    ctxT_out = nc.dram_tensor("ctxT", (HD, SEQ), F32, kind="ExternalOutput").ap()
    if ng:
        globplane_out = nc.dram_tensor("globplane", (ng, SEQ), F32, kind="ExternalOutput").ap()
        grow_out = nc.dram_tensor("growraw", (ng, SEQ), F32, kind="ExternalOutput").ap()
        pgst_out = nc.dram_tensor("pgst", (P, NT, ng), F32, kind="ExternalOutput").ap()
        gctxT_out = nc.dram_tensor("gctxT", (HD, ng), F32, kind="ExternalOutput").ap()

    uniform = False
    if ng >= 2:
        d = np.diff(globpos)
        uniform = len(set(d.tolist())) == 1
        g0, gst = int(globpos[0]), int(d[0])
    elif ng == 1:
        uniform, g0, gst = True, int(globpos[0]), 1

    with tile.TileContext(nc) as tc:
        with ExitStack() as ctx:
            consts = ctx.enter_context(tc.tile_pool(name="consts", bufs=1))

            # ---- load inputs; stage fp32, round to f32r on DVE ----
            staging = tc.tile_pool(name="staging", bufs=1)
            stg = staging.__enter__()
            id_sb = stg.tile([P, P], F32)
            band_sb = stg.tile([P, 3, P], F32)
            hs_h = consts.tile([P, KC, SEQ], F16)
            w_h = consts.tile([P, KC, 384], F16)
            for c in range(KC):
                nc.sync.dma_start(w_h[:, c], wpack[c * P:(c + 1) * P, :])
                nc.sync.dma_start(hs_h[:, c], hsT[c * P:(c + 1) * P, :])
            nc.gpsimd.dma_start(id_sb[:], ident)
            nc.gpsimd.dma_start(band_sb[:], band)
            bp_sb = consts.tile([P, 3], F32)
            nc.gpsimd.dma_start(bp_sb[:], bpack)
            biaspt_sb = consts.tile([P, NT], F32)
            nc.gpsimd.dma_start(biaspt_sb[:], biaspt)
            biasptg_sb = consts.tile([P, NT], F32)
            nc.gpsimd.dma_start(biasptg_sb[:], biasptg)
            gsec_sb = consts.tile([max(ng, 1), 1], F32)
            nc.gpsimd.dma_start(gsec_sb[:], gsec)

            id_r = consts.tile([P, P], F32R)
            nc.vector.tensor_copy(id_r[:], id_sb[:])
            band_r = consts.tile([P, 3, P], F32R)
            nc.vector.tensor_copy(band_r[:], band_sb[:])
            warm = consts.tile([P, 1], F32)
            nc.scalar.activation(warm[:], id_sb[:, 0:1],
                                 mybir.ActivationFunctionType.Exp, bias=0.0, scale=1.0)
            zeros_r = consts.tile([P, HD], F32R)
            nc.vector.tensor_scalar(out=zeros_r[:], in0=id_sb[:, 0:HD],
                                    scalar1=0.0, scalar2=None,
                                    op0=mybir.AluOpType.mult)
            staging.__exit__(None, None, None)

            # ---- projections + attention, phase-ordered for DMA overlap ----
            # pack0 = [q|qg], pack1 = [k|kg], pack2 = [v|vg]; q/qg pre-scaled.
            Exp = mybir.ActivationFunctionType.Exp
            proj = [consts.tile([P, SEQ], F32R, tag=f"proj{p}", name=f"proj{p}")
                    for p in range(3)]
            ps = ctx.enter_context(tc.tile_pool(name="ps", bufs=4, space="PSUM"))

            ps_pj_cm = tc.tile_pool(name="ps_pj", bufs=4, space="PSUM")
            ps_pj = ps_pj_cm.__enter__()

            def do_pack(p):
                psas = [ps_pj.tile([P, 512], F32, tag="pj", name="psa") for _ in range(NSC)]
                for c in range(KC):
                    for s in range(NSC):
                        nc.tensor.matmul(psas[s][:], w_h[:, c, p * P:(p + 1) * P],
                                         hs_h[:, c, s * 512:(s + 1) * 512],
                                         start=(c == 0), stop=(c == KC - 1),
                                         skip_group_check=True)
                for s in range(NSC):
                    nc.vector.tensor_scalar(
                        out=proj[p][:, s * 512:(s + 1) * 512], in0=psas[s][:],
                        scalar1=bp_sb[:, p:p + 1], scalar2=None,
                        op0=mybir.AluOpType.add)

            do_pack(0)
            do_pack(1)
            qT = proj[0][0:64, :]
            qgT = proj[0][64:128, :]
            kT = proj[1][0:64, :]
            kgT = proj[1][64:128, :]

            gcol_stages = []

            def glob_cols(src):
                """(64, SEQ) AP -> (64, ng) AP of the global key columns."""
                if uniform:
                    return src[:, g0:g0 + gst * ng:gst]
                st = consts.tile([64, ng], F32R, tag=f"gcols{len(gcol_stages)}",
                                 name=f"gcols{len(gcol_stages)}")
                gcol_stages.append(st)
                for i, pp in enumerate(globpos):
                    nc.vector.tensor_copy(st[:, i:i + 1].bitcast(F32),
                                          src[:, int(pp):int(pp) + 1].bitcast(F32))
                return st[:]

            # ---- band score planes ----
            planes = consts.tile([P, NT, 384], F32R)
            for k in range(NT):
                _, _, _, _, vlo, vhi = _plane_geom(k)
                if vlo > 0:
                    nc.vector.memset(planes[:, k, 0:vlo].bitcast(F32), 0.0)
                if vhi < 384:
                    nc.vector.memset(planes[:, k, vhi:384].bitcast(F32), 0.0)
            pfl = planes[:].rearrange("p n w -> p (n w)")
            ofl = planes_out.rearrange("p n w -> p (n w)")
            for k in range(NT):
                tmin, tmax, qlo, qhi, vlo, vhi = _plane_geom(k)
                spt = ps.tile([P, 384], F32, tag="sm", name="spt")
                nc.tensor.matmul(spt[:, vlo:vhi],
                                 kT[:, k * P:(k + 1) * P], qT[:, qlo:qhi],
                                 start=True, stop=False, skip_group_check=True)
                for t in range(tmin, tmax + 1):
                    if t == k:
                        continue  # diagonal block has zero band bias
                    w = (1 - k + t) * P
                    nc.tensor.matmul(spt[:, w:w + P].bitcast(F32R),
                                     band_r[:, 1 + (k - t), :], id_r[:],
                                     is_transpose=True, start=False,
                                     stop=(t == tmax or (t + 1 == k and k == tmax)),
                                     skip_group_check=True)
                nc.scalar.activation(planes[:, k, vlo:vhi], spt[:, vlo:vhi],
                                     Exp, bias=biaspt_sb[:, k:k + 1], scale=1.0)
                if k % 2 == 1:
                    nc.sync.dma_start(ofl[:, (k - 1) * 384:(k + 1) * 384],
                                      pfl[:, (k - 1) * 384:(k + 1) * 384].bitcast(F32))

            # ---- early dense passes at the ng global positions (overlap DMA) ----
            if ng:
                kglob = glob_cols(kT)
                qTg = glob_cols(qT)
                qgTg = glob_cols(qgT)
                if uniform:
                    kg_mm = kgT
                else:
                    kg0 = consts.tile([64, SEQ], F32R)
                    nc.vector.tensor_copy(kg0[:].bitcast(F32), kgT.bitcast(F32))
                    kg_mm = kg0[:]

                globplane = consts.tile([ng, SEQ], F32R)
                for s in range(NSC):
                    gpt = ps.tile([P, 512], F32, tag="sm", name="gpt")
                    nc.tensor.matmul(gpt[0:ng, :], kglob,
                                     qT[:, s * 512:(s + 1) * 512],
                                     start=True, stop=True)
                    nc.scalar.activation(globplane[:, s * 512:(s + 1) * 512],
                                         gpt[0:ng, :], Exp, bias=gsec_sb[:],
                                         scale=1.0)
                nc.sync.dma_start(globplane_out, globplane[:].bitcast(F32))

                growraw = consts.tile([ng, SEQ], F32R)
                for s in range(NSC):
                    grp = ps.tile([P, 512], F32, tag="sm", name="grp")
                    nc.tensor.matmul(grp[0:ng, :], qTg,
                                     kT[:, s * 512:(s + 1) * 512],
                                     start=True, stop=True)
                    nc.scalar.activation(growraw[:, s * 512:(s + 1) * 512],
                                         grp[0:ng, :], Exp, bias=gsec_sb[:],
                                         scale=1.0)
                nc.sync.dma_start(grow_out, growraw[:].bitcast(F32))

                pgst = consts.tile([P, NT, ng], F32R)
                for g in range(NT // 4):
                    pgp = ps.tile([P, 4 * ng], F32, tag="sm", name="pgp")
                    for i in range(4):
                        c = 4 * g + i
                        nc.tensor.matmul(pgp[:, i * ng:(i + 1) * ng],
                                         kg_mm[:, c * P:(c + 1) * P], qgTg,
                                         start=True, stop=True)
                    for (i0, ilen) in biasptg_runs[g]:
                        nc.scalar.activation(
                            pgst[:, 4 * g + i0:4 * g + i0 + ilen, :],
                            pgp[:, i0 * ng:(i0 + ilen) * ng], Exp,
                            bias=biasptg_sb[:, 4 * g + i0:4 * g + i0 + 1],
                            scale=1.0)
                nc.sync.dma_start(
                    pgst_out.rearrange("p n w -> p (n w)"),
                    pgst[:].rearrange("p n w -> p (n w)").bitcast(F32))

            # ---- pack2 (v|vg) + V/Vg in (keys, 64) layout ----
            do_pack(2)
            ps_pj_cm.__exit__(None, None, None)
            vT = proj[2][0:64, :]
            vgT = proj[2][64:128, :]
            v_r = consts.tile([P, NT, HD], F32R)
            vg_r = consts.tile([P, NT, HD], F32R)
            for g in range(NT // 4):
                vps = ps.tile([P, 4 * HD], F32R, tag="sm", name="vps")
                for i in range(4):
                    c = 4 * g + i
                    nc.tensor.transpose(vps[:, i * HD:(i + 1) * HD],
                                        vT[:, c * P:(c + 1) * P], id_r[0:64, 0:64])
                nc.vector.tensor_copy(v_r[:, 4 * g:4 * g + 4, :], vps[:])
            if ng:
                for g in range(NT // 4):
                    vps = ps.tile([P, 4 * HD], F32R, tag="sm", name="vps")
                    for i in range(4):
                        c = 4 * g + i
                        nc.tensor.transpose(vps[:, i * HD:(i + 1) * HD],
                                            vgT[:, c * P:(c + 1) * P],
                                            id_r[64:128, 64:128])
                    nc.vector.tensor_copy(vg_r[:, 4 * g:4 * g + 4, :], vps[:])
                vglob_r = consts.tile([ng, HD], F32R)
                vps = ps.tile([P, 4 * HD], F32R, tag="sm", name="vps")
                nc.tensor.transpose(vps[0:ng, 0:HD], glob_cols(vT), id_r[0:64, 0:64])
                nc.vector.tensor_copy(vglob_r[:], vps[0:ng, 0:HD])

            # ---- ctxT accumulation ----
            ctxT_sb = consts.tile([HD, SEQ], F32)
            with tc.tile_pool(name="ps_ctxT", bufs=1, space="PSUM") as ps_ctxT:
                ctps = ps_ctxT.tile([HD, SEQ], F32, tag="ctxT", name="ctps")
                segs = [[] for _ in range(NSC)]
                for k in range(NT):
                    tmin, tmax, qlo, qhi, vlo, vhi = _plane_geom(k)
                    cuts = [qlo] + [b for b in (512, 1024, 1536) if qlo < b < qhi] + [qhi]
                    for a, b in zip(cuts, cuts[1:]):
                        segs[a // 512].append((k, a, b, vlo + (a - qlo), vlo + (b - qlo)))
                for s in range(NSC):
                    lo, hi = s * 512, (s + 1) * 512
                    nc.tensor.matmul(ctps[:, lo:hi], zeros_r[:],
                                     proj[0][:, lo:hi],
                                     start=True, stop=False, skip_group_check=True)
                    for (k, a, b, pa, pb) in segs[s]:
                        nc.tensor.matmul(ctps[:, a:b], v_r[:, k, :],
                                         planes[:, k, pa:pb],
                                         start=False, stop=False, skip_group_check=True)
                    if ng:
                        nc.tensor.matmul(ctps[:, lo:hi], vglob_r[:],
                                         globplane[:, lo:hi],
                                         start=False, stop=True, skip_group_check=True)
                    else:
                        nc.tensor.matmul(ctps[:, lo:hi], zeros_r[:],
                                         proj[0][:, lo:hi],
                                         start=False, stop=True, skip_group_check=True)
                    nc.scalar.copy(ctxT_sb[:, lo:hi], ctps[:, lo:hi])
                    nc.sync.dma_start(ctxT_out[:, lo:hi], ctxT_sb[:, lo:hi])

                if ng:
                    gcps = ps.tile([HD, ng], F32, tag="sm", name="gcps")
                    for k in range(NT):
                        nc.tensor.matmul(gcps[:], vg_r[:, k, :], pgst[:, k, :],
                                         start=(k == 0), stop=(k == NT - 1))
                    gctxT_sb = consts.tile([HD, ng], F32)
                    nc.scalar.copy(gctxT_sb[:], gcps[:])
                    nc.sync.dma_start(gctxT_out, gctxT_sb[:])

    nc.compile()
    return nc


_PROGRAM_CACHE = {}


def _biasptg_runs(bias_ptg):
    """Per psum-group-of-4: runs of consecutive key tiles with identical
    pad-mask bias columns (each run shares one exp instruction)."""
    runs = []
    for g in range(NT // 4):
        r, i = [], 0
        while i < 4:
            j = i
            while (j + 1 < 4 and np.array_equal(bias_ptg[:, 4 * g + j + 1],
                                                bias_ptg[:, 4 * g + i])):
                j += 1
            r.append((i, j - i + 1))
            i = j + 1
        runs.append(r)
    return runs


def _get_program(ng, globpos, bias_ptg):
    runs = _biasptg_runs(bias_ptg)
    key = (ng, tuple(globpos.tolist()), bias_ptg.tobytes())
    if key not in _PROGRAM_CACHE:
        _PROGRAM_CACHE[key] = _build_program(ng, globpos, runs)
    return _PROGRAM_CACHE[key]


def _pack_weights(Wq, bq, Wk, bk, Wv, bv, Wqg, bqg, Wkg, bkg, Wvg, bvg, h):
    """Per-head packed weight (DIM, 384) and bias (P, 3), scale folded into q/qg."""
    sl = slice(h * HD, (h + 1) * HD)
    wpack = np.concatenate([Wq[sl].T * SCALE, Wqg[sl].T * SCALE,
                            Wk[sl].T, Wkg[sl].T,
                            Wv[sl].T, Wvg[sl].T], axis=1).astype(np.float16)
    bpack = np.stack([np.concatenate([bq[sl] * SCALE, bqg[sl] * SCALE]),
                      np.concatenate([bk[sl], bkg[sl]]),
                      np.concatenate([bv[sl], bvg[sl]])], axis=1)
    return np.ascontiguousarray(wpack), np.ascontiguousarray(bpack, np.float32)


def make_in_maps(hidden_states, Wq, bq, Wk, bk, Wv, bv, Wqg, bqg, Wkg, bkg, Wvg, bvg,
                 attn_mask):
    """Build (nc, in_maps, meta) for the SPMD run."""
    hs = np.asarray(hidden_states, np.float32)
    m = np.asarray(attn_mask).reshape(-1).astype(np.int64)
    assert hs.shape == (1, SEQ, DIM) and m.shape == (SEQ,)
    is_pad, globpos, ng, band_src, bias_pt, bias_ptg, gsec, notpad = _mask_tables(m)
    nc = _get_program(ng, globpos, bias_ptg)

    hsT = np.ascontiguousarray(hs[0].T.astype(np.float16))
    ident = np.eye(P, dtype=np.float32)
    shared = {"hsT": hsT, "band": band_src, "biaspt": bias_pt, "biasptg": bias_ptg,
              "gsec": gsec, "ident": ident}
    args = [np.asarray(a, np.float32) for a in
            (Wq, bq, Wk, bk, Wv, bv, Wqg, bqg, Wkg, bkg, Wvg, bvg)]
    in_maps = []
    for h in range(N_CORES):
        wpack, bpack = _pack_weights(*args, h)
        in_maps.append(dict(shared, wpack=wpack, bpack=bpack))
    meta = (globpos, ng, notpad)
    return nc, in_maps, meta


def assemble(results, meta):
    """Host: scatter planes into dense attn, normalize softmax, build out."""
    globpos, ng, notpad = meta
    attn = np.zeros((1, HEADS, SEQ, SEQ), np.float32)
    out = np.zeros((1, SEQ, DIM), np.float32)
    for h in range(N_CORES):
        r = results[h]
        A = attn[0, h]
        pl = r["planes"]                                    # (P, NT, 384)
        for k in range(NT):
            tmin, tmax, _, _, _, _ = _plane_geom(k)
            for t in range(tmin, tmax + 1):
                w = (1 - k + t) * P
                A[t * P:(t + 1) * P, k * P:(k + 1) * P] = pl[:, k, w:w + P].T
        if ng:
            A[:, globpos] = r["globplane"].T
        W = A.sum(axis=1)
        W[W == 0] = 1.0
        scl = notpad / W
        A *= scl[:, None]
        if ng:
            grow = r["growraw"] * notpad[None, :]
            gw = grow.sum(axis=1, keepdims=True)
            gw[gw == 0] = 1.0
            A[globpos, :] = grow / gw
        out[0, :, h * HD:(h + 1) * HD] = r["ctxT"].T * scl[:, None]
        if ng:
            wg = r["pgst"].sum(axis=(0, 1))                 # (ng,)
            wg[wg == 0] = 1.0
            out[0, globpos, h * HD:(h + 1) * HD] = r["gctxT"].T / wg[:, None]
    return out, attn


def kernel(hidden_states, Wq, bq, Wk, bk, Wv, bv, Wqg, bqg, Wkg, bkg, Wvg, bvg,
           attn_mask):
    nc, in_maps, meta = make_in_maps(hidden_states, Wq, bq, Wk, bk, Wv, bv,
                                     Wqg, bqg, Wkg, bkg, Wvg, bvg, attn_mask)
    res = run_bass_kernel_spmd(nc, in_maps, core_ids=list(range(N_CORES)))
    return assemble(res.results, meta)


# revision 23
# speedup vs baseline: 2.3183x; 1.0291x over previous
"""Trainium2 Bass kernel for nn_LocalGlobalAttention (band + global-token attention).

Sharding: tensor-parallel over the 8 heads — one head per NeuronCore, no
collectives. Host concatenates per-head results.

Device computes, per head (all in key-major "plane" layout):
  planes[k]   = exp(K_k^T Q_win + band_bias)   (128 keys x <=384 queries)
  globplane   = exp(Kglob^T Q)                 (ng global keys x 2048 queries)
  ctxT        = V^T-weighted accumulation over planes       (64 x 2048)
  growraw     = exp(Qglob^T K) for the ng global query rows  (ng x 2048)
  pgst/gctxT  = the global-attention pass at the ng global query positions
Softmax normalization (row sums, 1/W scaling, pad masking) happens on the
host, which also scatters the compact planes into the dense attention output.
Masked entries are exact zeros (bias -1e4 underflows exp to 0.0 in fp32),
so the banded+global sparsity is exact, not approximate.

Matmuls run as float32r (TF32-like, ~1e-4) with fp32 PSUM accumulation.
"""
import os
import sys
from contextlib import ExitStack

for _p in ("/opt/trn_rl_repo", "/root/.axon_site/_ro/trn_rl_repo"):
    if os.path.isdir(_p) and _p not in sys.path:
        sys.path.insert(0, _p)

import numpy as np

import concourse.bass as bass
import concourse.tile as tile
from concourse import bacc, mybir
from concourse.bass_utils import run_bass_kernel_spmd

SEQ = 2048
DIM = 512
HEADS = 8
HD = 64          # head dim
WIN = 128        # local attention window
NEG = -10000.0
P = 128          # partitions
NT = SEQ // P    # 16 key/query tiles
KC = DIM // P    # 4 contraction chunks
NSC = SEQ // 512  # 4 sequence chunks of 512
SCALE = 1.0 / np.sqrt(HD)
F32 = mybir.dt.float32
F32R = mybir.dt.float32r
F16 = mybir.dt.float16

N_CORES = 8


def _plane_geom(k):
    """Plane k covers queries of tiles k-1..k+1 at window offsets 0/128/256."""
    tmin, tmax = max(k - 1, 0), min(k + 1, NT - 1)
    qlo, qhi = tmin * P, (tmax + 1) * P
    vlo = (1 - k + tmin) * P
    vhi = (1 - k + tmax) * P + P
    return tmin, tmax, qlo, qhi, vlo, vhi


def _mask_tables(m):
    """Host-side mask preprocessing. m: (SEQ,) int (1 real, -1 global, 0 pad)."""
    is_pad = m == 0
    is_glob = m == -1
    globpos = np.flatnonzero(is_glob).astype(np.int64)
    ng = len(globpos)

    a = np.arange(P)
    band_src = np.zeros((P, 3, P), np.float32)
    band_src[:, 0, :] = np.where(a[:, None] <= a[None, :], 0.0, NEG)
    band_src[:, 2, :] = np.where(a[:, None] >= a[None, :], 0.0, NEG)

    dead = is_pad | is_glob                      # keys excluded from band planes
    bias_pt = np.where(dead.reshape(NT, P).T, NEG, 0.0).astype(np.float32)   # (P, NT)
    bias_ptg = np.where(is_pad.reshape(NT, P).T, NEG, 0.0).astype(np.float32)
    gsec = (np.where(is_pad[globpos], NEG, 0.0).astype(np.float32).reshape(ng, 1)
            if ng else np.zeros((1, 1), np.float32))
    notpad = (~is_pad).astype(np.float32)        # (SEQ,)
    return is_pad, globpos, ng, band_src, bias_pt, bias_ptg, gsec, notpad


def _build_program(ng, globpos, biasptg_runs):
    """Trace the per-core Bass/Tile program (SPMD: same program on all cores)."""
    Exp = mybir.ActivationFunctionType.Exp
    nc = bacc.Bacc("TRN2", target_bir_lowering=False, debug=False, num_devices=1)

    hsT = nc.dram_tensor("hsT", (DIM, SEQ), F16, kind="ExternalInput").ap()
    wpack = nc.dram_tensor("wpack", (DIM, 384), F16, kind="ExternalInput").ap()
    bpack = nc.dram_tensor("bpack", (P, 3), F32, kind="ExternalInput").ap()
    band = nc.dram_tensor("band", (P, 3, P), F32, kind="ExternalInput").ap()
    biaspt = nc.dram_tensor("biaspt", (P, NT), F32, kind="ExternalInput").ap()
    biasptg = nc.dram_tensor("biasptg", (P, NT), F32, kind="ExternalInput").ap()
    gsec = nc.dram_tensor("gsec", (max(ng, 1), 1), F32, kind="ExternalInput").ap()
    ident = nc.dram_tensor("ident", (P, P), F32, kind="ExternalInput").ap()

    planes_out = nc.dram_tensor("planes", (P, NT, 384), F16, kind="ExternalOutput").ap()
    ctxT_out = nc.dram_tensor("ctxT", (HD, SEQ), F32, kind="ExternalOutput").ap()
    if ng:
        globplane_out = nc.dram_tensor("globplane", (ng, SEQ), F16, kind="ExternalOutput").ap()
        grow_out = nc.dram_tensor("growraw", (ng, SEQ), F16, kind="ExternalOutput").ap()
        pgst_out = nc.dram_tensor("pgst", (P, NT, ng), F16, kind="ExternalOutput").ap()
        gctxT_out = nc.dram_tensor("gctxT", (HD, ng), F32, kind="ExternalOutput").ap()

    uniform = False
    if ng >= 2:
        d = np.diff(globpos)
        uniform = len(set(d.tolist())) == 1
        g0, gst = int(globpos[0]), int(d[0])
    elif ng == 1:
        uniform, g0, gst = True, int(globpos[0]), 1

    with tile.TileContext(nc) as tc:
        with ExitStack() as ctx:
            consts = ctx.enter_context(tc.tile_pool(name="consts", bufs=1))

            # ---- load inputs; stage fp32, round to f32r on DVE ----
            staging = tc.tile_pool(name="staging", bufs=1)
            stg = staging.__enter__()
            id_sb = stg.tile([P, P], F32)
            band_sb = stg.tile([P, 3, P], F32)
            hs_h = consts.tile([P, KC, SEQ], F16)
            w_h = consts.tile([P, KC, 384], F16)
            for c in range(KC):
                nc.sync.dma_start(w_h[:, c], wpack[c * P:(c + 1) * P, :])
                nc.sync.dma_start(hs_h[:, c, 0:1024], hsT[c * P:(c + 1) * P, 0:1024])
                nc.sync.dma_start(hs_h[:, c, 1024:2048], hsT[c * P:(c + 1) * P, 1024:2048])
            nc.gpsimd.dma_start(id_sb[:], ident)
            nc.gpsimd.dma_start(band_sb[:], band)
            bp_sb = consts.tile([P, 3], F32)
            nc.gpsimd.dma_start(bp_sb[:], bpack)
            biaspt_sb = consts.tile([P, NT], F32)
            nc.gpsimd.dma_start(biaspt_sb[:], biaspt)
            biasptg_sb = consts.tile([P, NT], F32)
            nc.gpsimd.dma_start(biasptg_sb[:], biasptg)
            gsec_sb = consts.tile([max(ng, 1), 1], F32)
            nc.gpsimd.dma_start(gsec_sb[:], gsec)

            id_r = consts.tile([P, P], F32R)
            nc.vector.tensor_copy(id_r[:], id_sb[:])
            band_r = consts.tile([P, 3, P], F32R)
            nc.vector.tensor_copy(band_r[:], band_sb[:])
            warm = consts.tile([P, 1], F32)
            nc.scalar.activation(warm[:], id_sb[:, 0:1],
                                 mybir.ActivationFunctionType.Exp, bias=0.0, scale=1.0)
            zeros_r = consts.tile([P, HD], F32R)
            nc.vector.tensor_scalar(out=zeros_r[:], in0=id_sb[:, 0:HD],
                                    scalar1=0.0, scalar2=None,
                                    op0=mybir.AluOpType.mult)
            staging.__exit__(None, None, None)

            # ---- projections + attention, phase-ordered for DMA overlap ----
            # pack0 = [q|qg], pack1 = [k|kg], pack2 = [v|vg]; q/qg pre-scaled.
            Exp = mybir.ActivationFunctionType.Exp
            proj = [consts.tile([P, SEQ], F32R, tag=f"proj{p}", name=f"proj{p}")
                    for p in range(3)]
            ps = ctx.enter_context(tc.tile_pool(name="ps", bufs=4, space="PSUM"))

            ps_pj_cm = tc.tile_pool(name="ps_pj", bufs=4, space="PSUM")
            ps_pj = ps_pj_cm.__enter__()

            def do_pack(p):
                psas = [ps_pj.tile([P, 512], F32, tag="pj", name="psa") for _ in range(NSC)]
                for c in range(KC):
                    for s in range(NSC):
                        nc.tensor.matmul(psas[s][:], w_h[:, c, p * P:(p + 1) * P],
                                         hs_h[:, c, s * 512:(s + 1) * 512],
                                         start=(c == 0), stop=(c == KC - 1),
                                         skip_group_check=True)
                for s in range(NSC):
                    nc.vector.tensor_scalar(
                        out=proj[p][:, s * 512:(s + 1) * 512], in0=psas[s][:],
                        scalar1=bp_sb[:, p:p + 1], scalar2=None,
                        op0=mybir.AluOpType.add)

            do_pack(0)
            do_pack(1)
            qT = proj[0][0:64, :]
            qgT = proj[0][64:128, :]
            kT = proj[1][0:64, :]
            kgT = proj[1][64:128, :]

            gcol_stages = []

            def glob_cols(src):
                """(64, SEQ) AP -> (64, ng) AP of the global key columns."""
                if uniform:
                    return src[:, g0:g0 + gst * ng:gst]
                st = consts.tile([64, ng], F32R, tag=f"gcols{len(gcol_stages)}",
                                 name=f"gcols{len(gcol_stages)}")
                gcol_stages.append(st)
                for i, pp in enumerate(globpos):
                    nc.vector.tensor_copy(st[:, i:i + 1].bitcast(F32),
                                          src[:, int(pp):int(pp) + 1].bitcast(F32))
                return st[:]

            # ---- band score planes ----
            planes = consts.tile([P, NT, 384], F16)
            for k in range(NT):
                _, _, _, _, vlo, vhi = _plane_geom(k)
                if vlo > 0:
                    nc.vector.memset(planes[:, k, 0:vlo], 0.0)
                if vhi < 384:
                    nc.vector.memset(planes[:, k, vhi:384], 0.0)
            pfl = planes[:].rearrange("p n w -> p (n w)")
            ofl = planes_out.rearrange("p n w -> p (n w)")
            for k in range(NT):
                tmin, tmax, qlo, qhi, vlo, vhi = _plane_geom(k)
                spt = ps.tile([P, 384], F32, tag="sm", name="spt")
                nc.tensor.matmul(spt[:, vlo:vhi],
                                 kT[:, k * P:(k + 1) * P], qT[:, qlo:qhi],
                                 start=True, stop=False, skip_group_check=True)
                for t in range(tmin, tmax + 1):
                    if t == k:
                        continue  # diagonal block has zero band bias
                    w = (1 - k + t) * P
                    nc.tensor.matmul(spt[:, w:w + P].bitcast(F32R),
                                     band_r[:, 1 + (k - t), :], id_r[:],
                                     is_transpose=True, start=False,
                                     stop=(t == tmax or (t + 1 == k and k == tmax)),
                                     skip_group_check=True)
                nc.scalar.activation(planes[:, k, vlo:vhi], spt[:, vlo:vhi],
                                     Exp, bias=biaspt_sb[:, k:k + 1], scale=1.0)
                if k % 2 == 1:
                    nc.sync.dma_start(ofl[:, (k - 1) * 384:(k + 1) * 384],
                                      pfl[:, (k - 1) * 384:(k + 1) * 384])

            # ---- early dense passes at the ng global positions (overlap DMA) ----
            if ng:
                kglob = glob_cols(kT)
                qTg = glob_cols(qT)
                qgTg = glob_cols(qgT)
                if uniform:
                    kg_mm = kgT
                else:
                    kg0 = consts.tile([64, SEQ], F32R)
                    nc.vector.tensor_copy(kg0[:].bitcast(F32), kgT.bitcast(F32))
                    kg_mm = kg0[:]

                globplane = consts.tile([ng, SEQ], F16)
                for s in range(NSC):
                    gpt = ps.tile([P, 512], F32, tag="sm", name="gpt")
                    nc.tensor.matmul(gpt[0:ng, :], kglob,
                                     qT[:, s * 512:(s + 1) * 512],
                                     start=True, stop=True)
                    nc.scalar.activation(globplane[:, s * 512:(s + 1) * 512],
                                         gpt[0:ng, :], Exp, bias=gsec_sb[:],
                                         scale=1.0)
                nc.sync.dma_start(globplane_out, globplane[:])

                growraw = consts.tile([ng, SEQ], F16)
                for s in range(NSC):
                    grp = ps.tile([P, 512], F32, tag="sm", name="grp")
                    nc.tensor.matmul(grp[0:ng, :], qTg,
                                     kT[:, s * 512:(s + 1) * 512],
                                     start=True, stop=True)
                    nc.scalar.activation(growraw[:, s * 512:(s + 1) * 512],
                                         grp[0:ng, :], Exp, bias=gsec_sb[:],
                                         scale=1.0)
                nc.sync.dma_start(grow_out, growraw[:])

                pgst = consts.tile([P, NT, ng], F16)
                for g in range(NT // 4):
                    pgp = ps.tile([P, 4 * ng], F32, tag="sm", name="pgp")
                    for i in range(4):
                        c = 4 * g + i
                        nc.tensor.matmul(pgp[:, i * ng:(i + 1) * ng],
                                         kg_mm[:, c * P:(c + 1) * P], qgTg,
                                         start=True, stop=True)
                    for (i0, ilen) in biasptg_runs[g]:
                        nc.scalar.activation(
                            pgst[:, 4 * g + i0:4 * g + i0 + ilen, :],
                            pgp[:, i0 * ng:(i0 + ilen) * ng], Exp,
                            bias=biasptg_sb[:, 4 * g + i0:4 * g + i0 + 1],
                            scale=1.0)
                nc.sync.dma_start(
                    pgst_out.rearrange("p n w -> p (n w)"),
                    pgst[:].rearrange("p n w -> p (n w)"))

            # ---- pack2 (v|vg) + V/Vg in (keys, 64) layout ----
            do_pack(2)
            ps_pj_cm.__exit__(None, None, None)
            vT = proj[2][0:64, :]
            vgT = proj[2][64:128, :]
            v_r = consts.tile([P, NT, HD], F16)
            vg_r = consts.tile([P, NT, HD], F16)
            for g in range(NT // 4):
                vps = ps.tile([P, 4 * HD], F32R, tag="sm", name="vps")
                for i in range(4):
                    c = 4 * g + i
                    nc.tensor.transpose(vps[:, i * HD:(i + 1) * HD],
                                        vT[:, c * P:(c + 1) * P], id_r[0:64, 0:64])
                nc.vector.tensor_copy(v_r[:, 4 * g:4 * g + 4, :], vps[:].bitcast(F32))
            if ng:
                for g in range(NT // 4):
                    vps = ps.tile([P, 4 * HD], F32R, tag="sm", name="vps")
                    for i in range(4):
                        c = 4 * g + i
                        nc.tensor.transpose(vps[:, i * HD:(i + 1) * HD],
                                            vgT[:, c * P:(c + 1) * P],
                                            id_r[64:128, 64:128])
                    nc.vector.tensor_copy(vg_r[:, 4 * g:4 * g + 4, :], vps[:].bitcast(F32))
                vglob_r = consts.tile([ng, HD], F16)
                vps = ps.tile([P, 4 * HD], F32R, tag="sm", name="vps")
                nc.tensor.transpose(vps[0:ng, 0:HD], glob_cols(vT), id_r[0:64, 0:64])
                nc.vector.tensor_copy(vglob_r[:], vps[0:ng, 0:HD].bitcast(F32))

            # ---- ctxT accumulation ----
            ctxT_sb = consts.tile([HD, SEQ], F32)
            with tc.tile_pool(name="ps_ctxT", bufs=1, space="PSUM") as ps_ctxT:
                ctps = ps_ctxT.tile([HD, SEQ], F32, tag="ctxT", name="ctps")
                segs = [[] for _ in range(NSC)]
                for k in range(NT):
                    tmin, tmax, qlo, qhi, vlo, vhi = _plane_geom(k)
                    cuts = [qlo] + [b for b in (512, 1024, 1536) if qlo < b < qhi] + [qhi]
                    for a, b in zip(cuts, cuts[1:]):
                        segs[a // 512].append((k, a, b, vlo + (a - qlo), vlo + (b - qlo)))
                for s in range(NSC):
                    lo, hi = s * 512, (s + 1) * 512
                    nc.tensor.matmul(ctps[:, lo:hi], zeros_r[:],
                                     proj[0][:, lo:hi],
                                     start=True, stop=False, skip_group_check=True)
                    for (k, a, b, pa, pb) in segs[s]:
                        nc.tensor.matmul(ctps[:, a:b], v_r[:, k, :],
                                         planes[:, k, pa:pb],
                                         start=False, stop=False, skip_group_check=True)
                    if ng:
                        nc.tensor.matmul(ctps[:, lo:hi], vglob_r[:],
                                         globplane[:, lo:hi],
                                         start=False, stop=True, skip_group_check=True)
                    else:
                        nc.tensor.matmul(ctps[:, lo:hi], zeros_r[:],
                                         proj[0][:, lo:hi],
                                         start=False, stop=True, skip_group_check=True)
                    nc.scalar.copy(ctxT_sb[:, lo:hi], ctps[:, lo:hi])
                    nc.sync.dma_start(ctxT_out[:, lo:hi], ctxT_sb[:, lo:hi])

                if ng:
                    gcps = ps.tile([HD, ng], F32, tag="sm", name="gcps")
                    for k in range(NT):
                        nc.tensor.matmul(gcps[:], vg_r[:, k, :], pgst[:, k, :],
                                         start=(k == 0), stop=(k == NT - 1))
                    gctxT_sb = consts.tile([HD, ng], F32)
                    nc.scalar.copy(gctxT_sb[:], gcps[:])
                    nc.sync.dma_start(gctxT_out, gctxT_sb[:])

    nc.compile()
    return nc


_PROGRAM_CACHE = {}


def _biasptg_runs(bias_ptg):
    """Per psum-group-of-4: runs of consecutive key tiles with identical
    pad-mask bias columns (each run shares one exp instruction)."""
    runs = []
    for g in range(NT // 4):
        r, i = [], 0
        while i < 4:
            j = i
            while (j + 1 < 4 and np.array_equal(bias_ptg[:, 4 * g + j + 1],
                                                bias_ptg[:, 4 * g + i])):
                j += 1
            r.append((i, j - i + 1))
            i = j + 1
        runs.append(r)
    return runs


def _get_program(ng, globpos, bias_ptg):
    runs = _biasptg_runs(bias_ptg)
    key = (ng, tuple(globpos.tolist()), bias_ptg.tobytes())
    if key not in _PROGRAM_CACHE:
        _PROGRAM_CACHE[key] = _build_program(ng, globpos, runs)
    return _PROGRAM_CACHE[key]


def _pack_weights(Wq, bq, Wk, bk, Wv, bv, Wqg, bqg, Wkg, bkg, Wvg, bvg, h):
    """Per-head packed weight (DIM, 384) and bias (P, 3), scale folded into q/qg."""
    sl = slice(h * HD, (h + 1) * HD)
    wpack = np.concatenate([Wq[sl].T * SCALE, Wqg[sl].T * SCALE,
                            Wk[sl].T, Wkg[sl].T,
                            Wv[sl].T, Wvg[sl].T], axis=1).astype(np.float16)
    bpack = np.stack([np.concatenate([bq[sl] * SCALE, bqg[sl] * SCALE]),
                      np.concatenate([bk[sl], bkg[sl]]),
                      np.concatenate([bv[sl], bvg[sl]])], axis=1)
    return np.ascontiguousarray(wpack), np.ascontiguousarray(bpack, np.float32)


def make_in_maps(hidden_states, Wq, bq, Wk, bk, Wv, bv, Wqg, bqg, Wkg, bkg, Wvg, bvg,
                 attn_mask):
    """Build (nc, in_maps, meta) for the SPMD run."""
    hs = np.asarray(hidden_states, np.float32)
    m = np.asarray(attn_mask).reshape(-1).astype(np.int64)
    assert hs.shape == (1, SEQ, DIM) and m.shape == (SEQ,)
    is_pad, globpos, ng, band_src, bias_pt, bias_ptg, gsec, notpad = _mask_tables(m)
    nc = _get_program(ng, globpos, bias_ptg)

    hsT = np.ascontiguousarray(hs[0].T.astype(np.float16))
    ident = np.eye(P, dtype=np.float32)
    shared = {"hsT": hsT, "band": band_src, "biaspt": bias_pt, "biasptg": bias_ptg,
              "gsec": gsec, "ident": ident}
    args = [np.asarray(a, np.float32) for a in
            (Wq, bq, Wk, bk, Wv, bv, Wqg, bqg, Wkg, bkg, Wvg, bvg)]
    in_maps = []
    for h in range(N_CORES):
        wpack, bpack = _pack_weights(*args, h)
        in_maps.append(dict(shared, wpack=wpack, bpack=bpack))
    meta = (globpos, ng, notpad)
    return nc, in_maps, meta


def assemble(results, meta):
    """Host: scatter planes into dense attn, normalize softmax, build out."""
    globpos, ng, notpad = meta
    attn = np.zeros((1, HEADS, SEQ, SEQ), np.float32)
    out = np.zeros((1, SEQ, DIM), np.float32)
    for h in range(N_CORES):
        r = results[h]
        A = attn[0, h]
        pl = r["planes"]                                    # (P, NT, 384)
        for k in range(NT):
            tmin, tmax, _, _, _, _ = _plane_geom(k)
            for t in range(tmin, tmax + 1):
                w = (1 - k + t) * P
                A[t * P:(t + 1) * P, k * P:(k + 1) * P] = pl[:, k, w:w + P].T
        if ng:
            A[:, globpos] = r["globplane"].T.astype(np.float32)
        W = A.sum(axis=1)
        W[W == 0] = 1.0
        scl = notpad / W
        A *= scl[:, None]
        if ng:
            grow = r["growraw"].astype(np.float32) * notpad[None, :]
            gw = grow.sum(axis=1, keepdims=True)
            gw[gw == 0] = 1.0
            A[globpos, :] = grow / gw
        out[0, :, h * HD:(h + 1) * HD] = r["ctxT"].T * scl[:, None]
        if ng:
            wg = r["pgst"].astype(np.float32).sum(axis=(0, 1))  # (ng,)
            wg[wg == 0] = 1.0
            out[0, globpos, h * HD:(h + 1) * HD] = r["gctxT"].T / wg[:, None]
    return out, attn


def kernel(hidden_states, Wq, bq, Wk, bk, Wv, bv, Wqg, bqg, Wkg, bkg, Wvg, bvg,
           attn_mask):
    nc, in_maps, meta = make_in_maps(hidden_states, Wq, bq, Wk, bk, Wv, bv,
                                     Wqg, bqg, Wkg, bkg, Wvg, bvg, attn_mask)
    res = run_bass_kernel_spmd(nc, in_maps, core_ids=list(range(N_CORES)))
    return assemble(res.results, meta)


# revision 24
# speedup vs baseline: 2.4351x; 1.0504x over previous
"""Trainium2 Bass kernel for nn_LocalGlobalAttention (band + global-token attention).

Sharding: tensor-parallel over the 8 heads — one head per NeuronCore, no
collectives. Host concatenates per-head results.

Device computes, per head (all in key-major "plane" layout):
  planes[k]   = exp(K_k^T Q_win + band_bias)   (128 keys x <=384 queries)
  globplane   = exp(Kglob^T Q)                 (ng global keys x 2048 queries)
  ctxT        = V^T-weighted accumulation over planes       (64 x 2048)
  growraw     = exp(Qglob^T K) for the ng global query rows  (ng x 2048)
  pgst/gctxT  = the global-attention pass at the ng global query positions
Softmax normalization (row sums, 1/W scaling, pad masking) happens on the
host, which also scatters the compact planes into the dense attention output.
Masked entries are exact zeros (bias -1e4 underflows exp to 0.0 in fp32),
so the banded+global sparsity is exact, not approximate.

Matmuls run as float32r (TF32-like, ~1e-4) with fp32 PSUM accumulation.
"""
import os
import sys
from contextlib import ExitStack

for _p in ("/opt/trn_rl_repo", "/root/.axon_site/_ro/trn_rl_repo"):
    if os.path.isdir(_p) and _p not in sys.path:
        sys.path.insert(0, _p)

import numpy as np

import concourse.bass as bass
import concourse.tile as tile
from concourse import bacc, mybir
from concourse.bass_utils import run_bass_kernel_spmd

SEQ = 2048
DIM = 512
HEADS = 8
HD = 64          # head dim
WIN = 128        # local attention window
NEG = -10000.0
P = 128          # partitions
NT = SEQ // P    # 16 key/query tiles
KC = DIM // P    # 4 contraction chunks
NSC = SEQ // 512  # 4 sequence chunks of 512
SCALE = 1.0 / np.sqrt(HD)
F32 = mybir.dt.float32
F32R = mybir.dt.float32r
F16 = mybir.dt.float16

N_CORES = 8


def _plane_geom(k):
    """Plane k covers queries of tiles k-1..k+1 at window offsets 0/128/256."""
    tmin, tmax = max(k - 1, 0), min(k + 1, NT - 1)
    qlo, qhi = tmin * P, (tmax + 1) * P
    vlo = (1 - k + tmin) * P
    vhi = (1 - k + tmax) * P + P
    return tmin, tmax, qlo, qhi, vlo, vhi


def _mask_tables(m):
    """Host-side mask preprocessing. m: (SEQ,) int (1 real, -1 global, 0 pad)."""
    is_pad = m == 0
    is_glob = m == -1
    globpos = np.flatnonzero(is_glob).astype(np.int64)
    ng = len(globpos)

    a = np.arange(P)
    band_src = np.zeros((P, 3, P), np.float32)
    band_src[:, 0, :] = np.where(a[:, None] <= a[None, :], 0.0, NEG)
    band_src[:, 2, :] = np.where(a[:, None] >= a[None, :], 0.0, NEG)

    dead = is_pad | is_glob                      # keys excluded from band planes
    bias_pt = np.where(dead.reshape(NT, P).T, NEG, 0.0).astype(np.float32)   # (P, NT)
    bias_ptg = np.where(is_pad.reshape(NT, P).T, NEG, 0.0).astype(np.float32)
    gsec = (np.where(is_pad[globpos], NEG, 0.0).astype(np.float32).reshape(ng, 1)
            if ng else np.zeros((1, 1), np.float32))
    notpad = (~is_pad).astype(np.float32)        # (SEQ,)
    return is_pad, globpos, ng, band_src, bias_pt, bias_ptg, gsec, notpad


def _build_program(ng, globpos, biasptg_runs):
    """Trace the per-core Bass/Tile program (SPMD: same program on all cores)."""
    Exp = mybir.ActivationFunctionType.Exp
    nc = bacc.Bacc("TRN2", target_bir_lowering=False, debug=False, num_devices=1)

    hsT = nc.dram_tensor("hsT", (DIM, SEQ), F16, kind="ExternalInput").ap()
    wpack = nc.dram_tensor("wpack", (DIM, 384), F16, kind="ExternalInput").ap()
    bpack = nc.dram_tensor("bpack", (P, 3), F32, kind="ExternalInput").ap()
    band = nc.dram_tensor("band", (P, 3, P), F32, kind="ExternalInput").ap()
    biaspt = nc.dram_tensor("biaspt", (P, NT), F32, kind="ExternalInput").ap()
    biasptg = nc.dram_tensor("biasptg", (P, NT), F32, kind="ExternalInput").ap()
    gsec = nc.dram_tensor("gsec", (max(ng, 1), 1), F32, kind="ExternalInput").ap()
    ident = nc.dram_tensor("ident", (P, P), F32, kind="ExternalInput").ap()

    planes_out = nc.dram_tensor("planes", (P, NT, 384), F16, kind="ExternalOutput").ap()
    ctxT_out = nc.dram_tensor("ctxT", (HD, SEQ), F32, kind="ExternalOutput").ap()
    if ng:
        globplane_out = nc.dram_tensor("globplane", (ng, SEQ), F16, kind="ExternalOutput").ap()
        grow_out = nc.dram_tensor("growraw", (ng, SEQ), F16, kind="ExternalOutput").ap()
        pgst_out = nc.dram_tensor("pgst", (P, NT, ng), F16, kind="ExternalOutput").ap()
        gctxT_out = nc.dram_tensor("gctxT", (HD, ng), F32, kind="ExternalOutput").ap()

    uniform = False
    if ng >= 2:
        d = np.diff(globpos)
        uniform = len(set(d.tolist())) == 1
        g0, gst = int(globpos[0]), int(d[0])
    elif ng == 1:
        uniform, g0, gst = True, int(globpos[0]), 1

    with tile.TileContext(nc) as tc:
        with ExitStack() as ctx:
            consts = ctx.enter_context(tc.tile_pool(name="consts", bufs=1))

            # ---- load inputs; stage fp32, round to f32r on DVE ----
            staging = tc.tile_pool(name="staging", bufs=1)
            stg = staging.__enter__()
            id_sb = stg.tile([P, P], F32)
            band_sb = stg.tile([P, 3, P], F32)
            hs_h = consts.tile([P, KC, SEQ], F16)
            w_h = consts.tile([P, KC, 384], F16)
            for c in range(KC):
                nc.sync.dma_start(w_h[:, c], wpack[c * P:(c + 1) * P, :])
                nc.sync.dma_start(hs_h[:, c, 0:1024], hsT[c * P:(c + 1) * P, 0:1024])
            for c in range(KC):
                nc.sync.dma_start(hs_h[:, c, 1024:2048], hsT[c * P:(c + 1) * P, 1024:2048])
            nc.gpsimd.dma_start(id_sb[:], ident)
            nc.gpsimd.dma_start(band_sb[:], band)
            bp_sb = consts.tile([P, 3], F32)
            nc.gpsimd.dma_start(bp_sb[:], bpack)
            biaspt_sb = consts.tile([P, NT], F32)
            nc.gpsimd.dma_start(biaspt_sb[:], biaspt)
            biasptg_sb = consts.tile([P, NT], F32)
            nc.gpsimd.dma_start(biasptg_sb[:], biasptg)
            gsec_sb = consts.tile([max(ng, 1), 1], F32)
            nc.gpsimd.dma_start(gsec_sb[:], gsec)

            id_r = consts.tile([P, P], F32R)
            nc.vector.tensor_copy(id_r[:], id_sb[:])
            band_r = consts.tile([P, 3, P], F32R)
            nc.vector.tensor_copy(band_r[:], band_sb[:])
            warm = consts.tile([P, 1], F32)
            nc.scalar.activation(warm[:], id_sb[:, 0:1],
                                 mybir.ActivationFunctionType.Exp, bias=0.0, scale=1.0)
            zeros_r = consts.tile([P, HD], F32R)
            nc.vector.tensor_scalar(out=zeros_r[:], in0=id_sb[:, 0:HD],
                                    scalar1=0.0, scalar2=None,
                                    op0=mybir.AluOpType.mult)
            staging.__exit__(None, None, None)

            # ---- projections + attention, phase-ordered for DMA overlap ----
            # pack0 = [q|qg], pack1 = [k|kg], pack2 = [v|vg]; q/qg pre-scaled.
            Exp = mybir.ActivationFunctionType.Exp
            proj = [consts.tile([P, SEQ], F32R, tag=f"proj{p}", name=f"proj{p}")
                    for p in range(3)]
            ps = ctx.enter_context(tc.tile_pool(name="ps", bufs=4, space="PSUM"))

            ps_pj_cm = tc.tile_pool(name="ps_pj", bufs=4, space="PSUM")
            ps_pj = ps_pj_cm.__enter__()

            def do_pack(p):
                psas = [ps_pj.tile([P, 512], F32, tag="pj", name="psa") for _ in range(NSC)]
                for c in range(KC):
                    for s in range(NSC):
                        nc.tensor.matmul(psas[s][:], w_h[:, c, p * P:(p + 1) * P],
                                         hs_h[:, c, s * 512:(s + 1) * 512],
                                         start=(c == 0), stop=(c == KC - 1),
                                         skip_group_check=True)
                for s in range(NSC):
                    nc.vector.tensor_scalar(
                        out=proj[p][:, s * 512:(s + 1) * 512], in0=psas[s][:],
                        scalar1=bp_sb[:, p:p + 1], scalar2=None,
                        op0=mybir.AluOpType.add)

            do_pack(0)
            do_pack(1)
            qT = proj[0][0:64, :]
            qgT = proj[0][64:128, :]
            kT = proj[1][0:64, :]
            kgT = proj[1][64:128, :]

            gcol_stages = []

            def glob_cols(src):
                """(64, SEQ) AP -> (64, ng) AP of the global key columns."""
                if uniform:
                    return src[:, g0:g0 + gst * ng:gst]
                st = consts.tile([64, ng], F32R, tag=f"gcols{len(gcol_stages)}",
                                 name=f"gcols{len(gcol_stages)}")
                gcol_stages.append(st)
                for i, pp in enumerate(globpos):
                    nc.vector.tensor_copy(st[:, i:i + 1].bitcast(F32),
                                          src[:, int(pp):int(pp) + 1].bitcast(F32))
                return st[:]

            # ---- band score planes ----
            planes = consts.tile([P, NT, 384], F16)
            for k in range(NT):
                _, _, _, _, vlo, vhi = _plane_geom(k)
                if vlo > 0:
                    nc.vector.memset(planes[:, k, 0:vlo], 0.0)
                if vhi < 384:
                    nc.vector.memset(planes[:, k, vhi:384], 0.0)
            pfl = planes[:].rearrange("p n w -> p (n w)")
            ofl = planes_out.rearrange("p n w -> p (n w)")
            for k in range(NT):
                tmin, tmax, qlo, qhi, vlo, vhi = _plane_geom(k)
                spt = ps.tile([P, 384], F32, tag="sm", name="spt")
                nc.tensor.matmul(spt[:, vlo:vhi],
                                 kT[:, k * P:(k + 1) * P], qT[:, qlo:qhi],
                                 start=True, stop=False, skip_group_check=True)
                for t in range(tmin, tmax + 1):
                    if t == k:
                        continue  # diagonal block has zero band bias
                    w = (1 - k + t) * P
                    nc.tensor.matmul(spt[:, w:w + P].bitcast(F32R),
                                     band_r[:, 1 + (k - t), :], id_r[:],
                                     is_transpose=True, start=False,
                                     stop=(t == tmax or (t + 1 == k and k == tmax)),
                                     skip_group_check=True)
                nc.scalar.activation(planes[:, k, vlo:vhi], spt[:, vlo:vhi],
                                     Exp, bias=biaspt_sb[:, k:k + 1], scale=1.0)
                if k % 2 == 1:
                    nc.sync.dma_start(ofl[:, (k - 1) * 384:(k + 1) * 384],
                                      pfl[:, (k - 1) * 384:(k + 1) * 384])

            # ---- early dense passes at the ng global positions (overlap DMA) ----
            if ng:
                kglob = glob_cols(kT)
                qTg = glob_cols(qT)
                qgTg = glob_cols(qgT)
                if uniform:
                    kg_mm = kgT
                else:
                    kg0 = consts.tile([64, SEQ], F32R)
                    nc.vector.tensor_copy(kg0[:].bitcast(F32), kgT.bitcast(F32))
                    kg_mm = kg0[:]

                globplane = consts.tile([ng, SEQ], F16)
                for s in range(NSC):
                    gpt = ps.tile([P, 512], F32, tag="sm", name="gpt")
                    nc.tensor.matmul(gpt[0:ng, :], kglob,
                                     qT[:, s * 512:(s + 1) * 512],
                                     start=True, stop=True)
                    nc.scalar.activation(globplane[:, s * 512:(s + 1) * 512],
                                         gpt[0:ng, :], Exp, bias=gsec_sb[:],
                                         scale=1.0)
                nc.sync.dma_start(globplane_out, globplane[:])

                growraw = consts.tile([ng, SEQ], F16)
                for s in range(NSC):
                    grp = ps.tile([P, 512], F32, tag="sm", name="grp")
                    nc.tensor.matmul(grp[0:ng, :], qTg,
                                     kT[:, s * 512:(s + 1) * 512],
                                     start=True, stop=True)
                    nc.scalar.activation(growraw[:, s * 512:(s + 1) * 512],
                                         grp[0:ng, :], Exp, bias=gsec_sb[:],
                                         scale=1.0)
                nc.sync.dma_start(grow_out, growraw[:])

                pgst = consts.tile([P, NT, ng], F16)
                for g in range(NT // 4):
                    pgp = ps.tile([P, 4 * ng], F32, tag="sm", name="pgp")
                    for i in range(4):
                        c = 4 * g + i
                        nc.tensor.matmul(pgp[:, i * ng:(i + 1) * ng],
                                         kg_mm[:, c * P:(c + 1) * P], qgTg,
                                         start=True, stop=True)
                    for (i0, ilen) in biasptg_runs[g]:
                        nc.scalar.activation(
                            pgst[:, 4 * g + i0:4 * g + i0 + ilen, :],
                            pgp[:, i0 * ng:(i0 + ilen) * ng], Exp,
                            bias=biasptg_sb[:, 4 * g + i0:4 * g + i0 + 1],
                            scale=1.0)
                nc.sync.dma_start(
                    pgst_out.rearrange("p n w -> p (n w)"),
                    pgst[:].rearrange("p n w -> p (n w)"))

            # ---- pack2 (v|vg) + V/Vg in (keys, 64) layout ----
            do_pack(2)
            ps_pj_cm.__exit__(None, None, None)
            vT = proj[2][0:64, :]
            vgT = proj[2][64:128, :]
            v_r = consts.tile([P, NT, HD], F16)
            vg_r = consts.tile([P, NT, HD], F16)
            for g in range(NT // 4):
                vps = ps.tile([P, 4 * HD], F32R, tag="sm", name="vps")
                for i in range(4):
                    c = 4 * g + i
                    nc.tensor.transpose(vps[:, i * HD:(i + 1) * HD],
                                        vT[:, c * P:(c + 1) * P], id_r[0:64, 0:64])
                nc.vector.tensor_copy(v_r[:, 4 * g:4 * g + 4, :], vps[:].bitcast(F32))
            if ng:
                for g in range(NT // 4):
                    vps = ps.tile([P, 4 * HD], F32R, tag="sm", name="vps")
                    for i in range(4):
                        c = 4 * g + i
                        nc.tensor.transpose(vps[:, i * HD:(i + 1) * HD],
                                            vgT[:, c * P:(c + 1) * P],
                                            id_r[64:128, 64:128])
                    nc.vector.tensor_copy(vg_r[:, 4 * g:4 * g + 4, :], vps[:].bitcast(F32))
                vglob_r = consts.tile([ng, HD], F16)
                vps = ps.tile([P, 4 * HD], F32R, tag="sm", name="vps")
                nc.tensor.transpose(vps[0:ng, 0:HD], glob_cols(vT), id_r[0:64, 0:64])
                nc.vector.tensor_copy(vglob_r[:], vps[0:ng, 0:HD].bitcast(F32))

            # ---- ctxT accumulation ----
            ctxT_sb = consts.tile([HD, SEQ], F32)
            with tc.tile_pool(name="ps_ctxT", bufs=1, space="PSUM") as ps_ctxT:
                ctps = ps_ctxT.tile([HD, SEQ], F32, tag="ctxT", name="ctps")
                segs = [[] for _ in range(NSC)]
                for k in range(NT):
                    tmin, tmax, qlo, qhi, vlo, vhi = _plane_geom(k)
                    cuts = [qlo] + [b for b in (512, 1024, 1536) if qlo < b < qhi] + [qhi]
                    for a, b in zip(cuts, cuts[1:]):
                        segs[a // 512].append((k, a, b, vlo + (a - qlo), vlo + (b - qlo)))
                for s in range(NSC):
                    lo, hi = s * 512, (s + 1) * 512
                    nc.tensor.matmul(ctps[:, lo:hi], zeros_r[:],
                                     proj[0][:, lo:hi],
                                     start=True, stop=False, skip_group_check=True)
                    for (k, a, b, pa, pb) in segs[s]:
                        nc.tensor.matmul(ctps[:, a:b], v_r[:, k, :],
                                         planes[:, k, pa:pb],
                                         start=False, stop=False, skip_group_check=True)
                    if ng:
                        nc.tensor.matmul(ctps[:, lo:hi], vglob_r[:],
                                         globplane[:, lo:hi],
                                         start=False, stop=True, skip_group_check=True)
                    else:
                        nc.tensor.matmul(ctps[:, lo:hi], zeros_r[:],
                                         proj[0][:, lo:hi],
                                         start=False, stop=True, skip_group_check=True)
                    nc.scalar.copy(ctxT_sb[:, lo:hi], ctps[:, lo:hi])
                    nc.sync.dma_start(ctxT_out[:, lo:hi], ctxT_sb[:, lo:hi])

                if ng:
                    gcps = ps.tile([HD, ng], F32, tag="sm", name="gcps")
                    for k in range(NT):
                        nc.tensor.matmul(gcps[:], vg_r[:, k, :], pgst[:, k, :],
                                         start=(k == 0), stop=(k == NT - 1))
                    gctxT_sb = consts.tile([HD, ng], F32)
                    nc.scalar.copy(gctxT_sb[:], gcps[:])
                    nc.sync.dma_start(gctxT_out, gctxT_sb[:])

    nc.compile()
    return nc


_PROGRAM_CACHE = {}


def _biasptg_runs(bias_ptg):
    """Per psum-group-of-4: runs of consecutive key tiles with identical
    pad-mask bias columns (each run shares one exp instruction)."""
    runs = []
    for g in range(NT // 4):
        r, i = [], 0
        while i < 4:
            j = i
            while (j + 1 < 4 and np.array_equal(bias_ptg[:, 4 * g + j + 1],
                                                bias_ptg[:, 4 * g + i])):
                j += 1
            r.append((i, j - i + 1))
            i = j + 1
        runs.append(r)
    return runs


def _get_program(ng, globpos, bias_ptg):
    runs = _biasptg_runs(bias_ptg)
    key = (ng, tuple(globpos.tolist()), bias_ptg.tobytes())
    if key not in _PROGRAM_CACHE:
        _PROGRAM_CACHE[key] = _build_program(ng, globpos, runs)
    return _PROGRAM_CACHE[key]


def _pack_weights(Wq, bq, Wk, bk, Wv, bv, Wqg, bqg, Wkg, bkg, Wvg, bvg, h):
    """Per-head packed weight (DIM, 384) and bias (P, 3), scale folded into q/qg."""
    sl = slice(h * HD, (h + 1) * HD)
    wpack = np.concatenate([Wq[sl].T * SCALE, Wqg[sl].T * SCALE,
                            Wk[sl].T, Wkg[sl].T,
                            Wv[sl].T, Wvg[sl].T], axis=1).astype(np.float16)
    bpack = np.stack([np.concatenate([bq[sl] * SCALE, bqg[sl] * SCALE]),
                      np.concatenate([bk[sl], bkg[sl]]),
                      np.concatenate([bv[sl], bvg[sl]])], axis=1)
    return np.ascontiguousarray(wpack), np.ascontiguousarray(bpack, np.float32)


def make_in_maps(hidden_states, Wq, bq, Wk, bk, Wv, bv, Wqg, bqg, Wkg, bkg, Wvg, bvg,
                 attn_mask):
    """Build (nc, in_maps, meta) for the SPMD run."""
    hs = np.asarray(hidden_states, np.float32)
    m = np.asarray(attn_mask).reshape(-1).astype(np.int64)
    assert hs.shape == (1, SEQ, DIM) and m.shape == (SEQ,)
    is_pad, globpos, ng, band_src, bias_pt, bias_ptg, gsec, notpad = _mask_tables(m)
    nc = _get_program(ng, globpos, bias_ptg)

    hsT = np.ascontiguousarray(hs[0].T.astype(np.float16))
    ident = np.eye(P, dtype=np.float32)
    shared = {"hsT": hsT, "band": band_src, "biaspt": bias_pt, "biasptg": bias_ptg,
              "gsec": gsec, "ident": ident}
    args = [np.asarray(a, np.float32) for a in
            (Wq, bq, Wk, bk, Wv, bv, Wqg, bqg, Wkg, bkg, Wvg, bvg)]
    in_maps = []
    for h in range(N_CORES):
        wpack, bpack = _pack_weights(*args, h)
        in_maps.append(dict(shared, wpack=wpack, bpack=bpack))
    meta = (globpos, ng, notpad)
    return nc, in_maps, meta


def assemble(results, meta):
    """Host: scatter planes into dense attn, normalize softmax, build out."""
    globpos, ng, notpad = meta
    attn = np.zeros((1, HEADS, SEQ, SEQ), np.float32)
    out = np.zeros((1, SEQ, DIM), np.float32)
    for h in range(N_CORES):
        r = results[h]
        A = attn[0, h]
        pl = r["planes"]                                    # (P, NT, 384)
        for k in range(NT):
            tmin, tmax, _, _, _, _ = _plane_geom(k)
            for t in range(tmin, tmax + 1):
                w = (1 - k + t) * P
                A[t * P:(t + 1) * P, k * P:(k + 1) * P] = pl[:, k, w:w + P].T
        if ng:
            A[:, globpos] = r["globplane"].T.astype(np.float32)
        W = A.sum(axis=1)
        W[W == 0] = 1.0
        scl = notpad / W
        A *= scl[:, None]
        if ng:
            grow = r["growraw"].astype(np.float32) * notpad[None, :]
            gw = grow.sum(axis=1, keepdims=True)
            gw[gw == 0] = 1.0
            A[globpos, :] = grow / gw
        out[0, :, h * HD:(h + 1) * HD] = r["ctxT"].T * scl[:, None]
        if ng:
            wg = r["pgst"].astype(np.float32).sum(axis=(0, 1))  # (ng,)
            wg[wg == 0] = 1.0
            out[0, globpos, h * HD:(h + 1) * HD] = r["gctxT"].T / wg[:, None]
    return out, attn


def kernel(hidden_states, Wq, bq, Wk, bk, Wv, bv, Wqg, bqg, Wkg, bkg, Wvg, bvg,
           attn_mask):
    nc, in_maps, meta = make_in_maps(hidden_states, Wq, bq, Wk, bk, Wv, bv,
                                     Wqg, bqg, Wkg, bkg, Wvg, bvg, attn_mask)
    res = run_bass_kernel_spmd(nc, in_maps, core_ids=list(range(N_CORES)))
    return assemble(res.results, meta)
